# revision 1
# baseline (speedup 1.0000x reference)
"""IPA (invariant point attention) Trainium2 kernel, 8-core SPMD, query-row sharded.

v3: flash-style pipeline with transposed logits, all-on-chip rotation.
  - pair host-transposed to [c, block, m, n_within] fp8(e4m3); Wpb bf16
  - bias matmuls produce [n_block, 12h] tiles (n on partitions), staged per
    block in bf16 (h-major); logits computed transposed L^T[n, m] via one
    matmul + an identity-matmul that accumulates the staged bias into the
    same PSUM group; exp gives E^T directly; attn@v accumulates [45ch, m]
    in PSUM across blocks with a ones-channel (channel 0) giving softmax
    denominators; normalized at the end via a reciprocal outer-product
  - point projections emit rows padded to 32-row spacing per head and
    duplicated per rotation coordinate (h,e,p), so rotation = row-wise
    multiply-add against per-row broadcast tensors, and rotated features
    land in ke64/qe64 via 32-aligned on-chip DVE copies (no DRAM roundtrip)
  - v projections emitted token-major directly; v-point rotation via fused
    scalar_tensor_tensor with per-token rot scalars on partitions
  - q2 term dropped (softmax-invariant); pair-bias bias bpb dropped
    (softmax-invariant); bv folded into bo_col; scale folded into weights
"""
import os
import sys
sys.path.insert(0, '/opt/trn_rl_repo')

import numpy as np
import ml_dtypes

_ABLATE = set(os.environ.get("IPA_ABLATE", "").split(","))
_REPEAT = int(os.environ.get("IPA_REPEAT", "1"))

import concourse.bass as bass
import concourse.mybir as mybir
from concourse.tile import TileContext
from concourse.vector_clock import ScopedClock
from concourse.bass_utils import run_bass_kernel_spmd

F32 = mybir.dt.float32
F16 = mybir.dt.float16
BF16 = mybir.dt.bfloat16
F8 = mybir.dt.float8e4
BF = ml_dtypes.bfloat16
E4M3 = ml_dtypes.float8_e4m3

N = 1024
M = 128
C = 384
H = 12
NB = 8          # n blocks
BL = 128        # block length
NCORES = 8
SCALE = (C // H) ** -0.5
EPS = 1e-5
Exp = mybir.ActivationFunctionType.Exp
Identity = mybir.ActivationFunctionType.Identity
Sqrt = mybir.ActivationFunctionType.Sqrt
Square = mybir.ActivationFunctionType.Square
ADD = mybir.AluOpType.add
MULT = mybir.AluOpType.mult

# WBLOB column offsets (per 128-row c-chunk)
WQ_O, WK_O, WKP_O, WQP_O, WVC_O = 0, 384, 768, 1920, 3072
WCH = 3600
# MISC column offsets
IDN_O, QEI_O, BSCK_O, BSCQ_O, GAM_O, BET_O, SM_O, WCAT_O, BO_O = \
    0, 128, 896, 908, 920, 1304, 1688, 2072, 4376
MISC_W = 4379
# KBLOB (per g, 128 rows): rb 3*1024 | tb 1024 | sel 12
KRB_O, KTB_O, KSEL_O = 0, 3072, 4096
KBW = 4108
# QBLOB (per g, 128 rows): rb 3*128 | tb 128
QRB_O, QTB_O = 0, 384
QBW = 512
# VBLOB: rot9 (8*9) | tbv (8*144)
VROT_O, VTBV_O = 0, 72
VBW = 72 + 1152

_MAXW = 1


def _patched_drain_and_barrier(self, tick_clock, wait_clock):
    # walrus rejects >2 sync waits on one Drain; split tail waits across nops
    nc = self.nc
    probe = nc.sync.nop()
    wait_clock.add_sem_waits(probe.ins, ScopedClock({None: tick_clock.global_clock}))
    waits = list(probe.ins.sync_info.on_wait or [])
    probe.ins.sync_info.on_wait = waits[:_MAXW]
    rest = waits[_MAXW:]
    while rest:
        n2 = nc.sync.nop()
        n2.ins.sync_info = mybir.SyncInfo(on_wait=rest[:_MAXW], on_update=[])
        rest = rest[_MAXW:]
    nc.sync.drain()
    nc.all_engine_barrier()
    assert self.sems is not None
    popped = nc._tile_sem_poison_stack.pop()
    assert popped is self._sem_poison
    nc.clear_and_free_semaphores(list(self.sems.allocated().values()))
    nc.all_engine_barrier()


TileContext._drain_and_barrier = _patched_drain_and_barrier

_orig_lower_ordered = TileContext._lower_ordered_insts


def _split_waits_then_lower(self, ordered):
    # HW instructions encode a limited number of sync waits; hoist excess
    # onto NoOps inserted immediately before, on the same engine.
    nc = self.nc
    for bb in list(ordered.keys()):
        insts = ordered[bb]
        new = []
        for inst in insts:
            si = getattr(inst, "sync_info", None)
            if si is not None and si.on_wait and len(si.on_wait) > _MAXW:
                waits = list(si.on_wait)
                while len(waits) > _MAXW:
                    chunk, waits = waits[:_MAXW], waits[_MAXW:]
                    nop = mybir.InstNoOp(
                        name=nc.get_next_instruction_name(),
                        engine=inst.engine, ins=[], outs=[], bass_nofuse=True,
                        sync_info=mybir.SyncInfo(on_wait=chunk, on_update=[]))
                    new.append(nop)
                si.on_wait = waits
            new.append(inst)
        ordered[bb] = new
    return _orig_lower_ordered(self, ordered)


TileContext._lower_ordered_insts = _split_waits_then_lower


def _emit(nc, pairT8, WB, SF, MISC, VB, KB, QB, PB, WPB, OUT):
    with TileContext(nc) as tc:
        with tc.tile_pool(name="persist", bufs=1) as pp, \
             tc.tile_pool(name="pair", bufs=2) as pairp, \
             tc.tile_pool(name="dstage", bufs=1, space="DRAM") as dstp:
            qe64 = pp.tile([128, 6 * M], BF16)
            ke64 = pp.tile([128, 6 * N], BF16)
            v_nat = pp.tile([128, NB * 540], BF16)
            stage = pp.tile([128, NB * 1536], BF16)
            wpb_sb = pp.tile([128, H + 128], BF16)
            misc = pp.tile([128, MISC_W], F32)
            vb_sb = pp.tile([128, VBW], F32)
            ones44 = pp.tile([1, 64], F32)
            cat_sb = pp.tile([128, 6 * 128], F32)
            k2sb = pp.tile([H, N], F32)
            k2_dram = dstp.tile([H, N], F32)

            nc.sync.dma_start(wpb_sb[:], WPB[:])
            nc.sync.dma_start(misc[:], MISC[:])
            nc.sync.dma_start(vb_sb[:], VB[:])
            nc.vector.memset(ke64[:], 0.0)
            nc.vector.memset(ones44[:], 1.0)
            nc.vector.memset(cat_sb[:], 0.0)
            nc.vector.tensor_copy(qe64[:], misc[:, QEI_O:QEI_O + 768])
            v_nat_v = v_nat[:].rearrange("p (j h c) -> p j h c", j=NB, h=H)
            nc.vector.memset(v_nat_v[:, :, :, 0:1], 1.0)

            idn_sb = misc[:, IDN_O:IDN_O + 128]
            idnb = wpb_sb[:, H:H + 128]

            # ====== phase A ======
            with tc.tile_pool(name="wload", bufs=1) as wl, \
                 tc.tile_pool(name="rot", bufs=2) as rp:
                wb = wl.tile([128, 3 * WCH], BF16)
                nc.sync.dma_start(wb[:], WB[:])
                sf = wl.tile([128, 3 * (N + M)], BF16)
                nc.sync.dma_start(sf[:], SF[:])
                kb = []
                qb = []
                pb = []
                for g in range(3):
                    t = wl.tile([128, KBW], BF16, tag=f"kb{g}", name=f"kb{g}")
                    nc.sync.dma_start(t[:], KB[g][:])
                    kb.append(t)
                    t = wl.tile([128, QBW], BF16, tag=f"qb{g}", name=f"qb{g}")
                    nc.sync.dma_start(t[:], QB[g][:])
                    qb.append(t)
                    t = wl.tile([128, 6], F32, tag=f"pb{g}", name=f"pb{g}")
                    nc.sync.dma_start(t[:], PB[g][:])
                    pb.append(t)
                kpT3 = [wl.tile([128, 3 * N], BF16, tag=f"kpT{g}", name=f"kpT{g}")
                        for g in range(3)]
                qpT3 = [wl.tile([128, 3 * M], BF16, tag=f"qpT{g}", name=f"qpT{g}")
                        for g in range(3)]
                vtmp = wl.tile([128, NB * 144], F32)

                def sfT(ci, o, w):
                    return sf[:, ci * (N + M) + o: ci * (N + M) + o + w]

                def smT(ci, o, w):
                    return sf[:, ci * (N + M) + N + o: ci * (N + M) + N + o + w]

                with tc.tile_pool(name="p1ps", bufs=2, space="PSUM") as pps, \
                     tc.tile_pool(name="vps", bufs=2, space="PSUM") as vps, \
                     tc.tile_pool(name="k2ps", bufs=1, space="PSUM") as k2pool:
                    # ---- scalar q/k projections (feature-major into ke64/qe64) ----
                    def grp_scal(wo, dste, bsc_o, mov, width, co, nb):
                        o = nb * 512
                        w = min(512, width - o)
                        ps = pps.tile([128, 512], F32, tag="proj", name="ps")
                        for ci in range(3):
                            nc.tensor.matmul(
                                ps[:, 0:w],
                                wb[:, ci * WCH + wo + co * 128: ci * WCH + wo + co * 128 + 128],
                                mov(ci, o, w),
                                start=(ci == 0), stop=(ci == 2))
                        for hh in range(4):
                            h = 4 * co + hh
                            t, u = h // 2, h % 2
                            nc.scalar.activation(
                                dste[64 * u:64 * u + 32, t * width + o: t * width + o + w],
                                ps[32 * hh:32 * hh + 32, 0:w], Identity,
                                bias=misc[64 * u:64 * u + 32, bsc_o + h:bsc_o + h + 1])

                    for co in range(3):
                        for nb in range(2):
                            grp_scal(WK_O, ke64, BSCK_O, sfT, N, co, nb)
                    for co in range(3):
                        grp_scal(WQ_O, qe64, BSCQ_O, smT, M, co, 0)

                    # ---- point projections, rows 32*hh + 4e + p per g of 4 heads ----
                    def grp_pt3(wo, dst, pbt, dcol, mov, width, g, d, nb):
                        o = nb * 512
                        w = min(512, width - o)
                        ps = pps.tile([128, 512], F32, tag="proj", name="ps")
                        for ci in range(3):
                            nc.tensor.matmul(
                                ps[:, 0:w],
                                wb[:, ci * WCH + wo + d * 384 + 128 * g:
                                   ci * WCH + wo + d * 384 + 128 * g + 128],
                                mov(ci, o, w),
                                start=(ci == 0), stop=(ci == 2))
                        nc.scalar.activation(
                            dst[:, d * width + o: d * width + o + w],
                            ps[:, 0:w], Identity, bias=pbt[:, dcol + d:dcol + d + 1])

                    for g in range(3):
                        for d in range(3):
                            for nb in range(2):
                                grp_pt3(WKP_O, kpT3[g], pb[g], 0, sfT, N, g, d, nb)
                    for g in range(3):
                        for d in range(3):
                            grp_pt3(WQP_O, qpT3[g], pb[g], 3, smT, M, g, d, 0)

                    # ---- v projections: token-major direct ----
                    for j in range(NB):
                        vs_ps = vps.tile([128, 384], F32, tag="vs", name="vs_ps")
                        vp_ps = vps.tile([128, 144], F32, tag="vp", name="vp_ps")
                        for ci in range(3):
                            nc.tensor.matmul(
                                vs_ps[:],
                                sfT(ci, j * BL, BL),
                                wb[:, ci * WCH + WVC_O: ci * WCH + WVC_O + 384],
                                start=(ci == 0), stop=(ci == 2))
                            nc.tensor.matmul(
                                vp_ps[:],
                                sfT(ci, j * BL, BL),
                                wb[:, ci * WCH + WVC_O + 384: ci * WCH + WVC_O + 528],
                                start=(ci == 0), stop=(ci == 2))
                        nc.scalar.copy(
                            v_nat_v[:, j, :, 1:33],
                            vs_ps[:].rearrange("p (h c) -> p h c", h=H))
                        nc.scalar.copy(vtmp[:, 144 * j:144 * (j + 1)], vp_ps[:])

                    # ---- k rotation on-chip: rc = sum_d kp_d * rb_d + tb ----
                    RCH = 512
                    k2t = [k2pool.tile([H, 512], F32, tag=f"k2_{half}", name=f"k2t{half}")
                           for half in range(2)]

                    def rot_chain(src, blob, rbo, tbo, width, o, w, tag):
                        rc = rp.tile([128, RCH], BF16, tag=f"rc{tag}", name="rc")
                        t2 = rp.tile([128, RCH], BF16, tag=f"rt{tag}", name="t2")
                        nc.vector.tensor_tensor(
                            rc[:, 0:w], src[:, o:o + w],
                            blob[:, rbo + o:rbo + o + w], MULT)
                        nc.vector.tensor_tensor(
                            t2[:, 0:w], src[:, width + o:width + o + w],
                            blob[:, rbo + width + o:rbo + width + o + w], MULT)
                        nc.vector.tensor_tensor(rc[:, 0:w], rc[:, 0:w], t2[:, 0:w], ADD)
                        nc.vector.tensor_tensor(
                            t2[:, 0:w], src[:, 2 * width + o:2 * width + o + w],
                            blob[:, rbo + 2 * width + o:rbo + 2 * width + o + w], MULT)
                        nc.vector.tensor_tensor(rc[:, 0:w], rc[:, 0:w], t2[:, 0:w], ADD)
                        nc.vector.tensor_tensor(
                            rc[:, 0:w], rc[:, 0:w],
                            blob[:, tbo + o:tbo + o + w], ADD)
                        return rc

                    for g in range(3):
                        for ci_ in range(2):
                            o = ci_ * RCH
                            rc = rot_chain(kpT3[g], kb[g], KRB_O, KTB_O, N, o, RCH, "k")
                            sq = rp.tile([128, RCH], BF16, tag="sq", name="sq")
                            nc.vector.tensor_tensor(sq[:], rc[:], rc[:], MULT)
                            nc.tensor.matmul(
                                k2t[ci_][:], kb[g][:, KSEL_O:KSEL_O + 12], sq[:],
                                start=(g == 0), stop=(g == 2))
                            for hh in range(4):
                                h = 4 * g + hh
                                t, u = h // 2, h % 2
                                nc.vector.tensor_copy(
                                    ke64[64 * u + 32:64 * u + 44, t * N + o: t * N + o + RCH],
                                    rc[32 * hh:32 * hh + 12, :])
                        qrc = rot_chain(qpT3[g], qb[g], QRB_O, QTB_O, M, 0, M, "q")
                        for hh in range(4):
                            h = 4 * g + hh
                            t, u = h // 2, h % 2
                            nc.vector.tensor_copy(
                                qe64[64 * u + 32:64 * u + 44, t * M:(t + 1) * M],
                                qrc[32 * hh:32 * hh + 12, 0:M])

                    # k2 rows into ke64 (partition scatter via DRAM roundtrip)
                    for half in range(2):
                        nc.vector.tensor_copy(
                            k2sb[:, half * 512:(half + 1) * 512], k2t[half][:])
                    nc.gpsimd.dma_start(k2_dram[:], k2sb[:])
                    for u in range(2):
                        dst = ke64[64 * u + 44: 64 * u + 45, :].rearrange(
                            "one (t n) -> one t n", t=6)
                        src = k2_dram[:].rearrange("(t u2) n -> u2 t n", u2=2)[u:u + 1]
                        nc.gpsimd.dma_start(dst, src)

                    # ---- v-point rotation via per-token scalars ----
                    rot9 = vb_sb[:, VROT_O:VROT_O + 72]
                    tbv = vb_sb[:, VTBV_O:VTBV_O + 1152]
                    for j in range(NB):
                        for e in range(3):
                            t1 = rp.tile([128, 48], F32, tag="vst1", name="t1")
                            t2 = rp.tile([128, 48], F32, tag="vst2", name="t2")
                            nc.vector.scalar_tensor_tensor(
                                t1[:], vtmp[:, 144 * j:144 * j + 48],
                                rot9[:, 9 * j + e:9 * j + e + 1],
                                tbv[:, 144 * j + 48 * e: 144 * j + 48 * e + 48], MULT, ADD)
                            nc.vector.scalar_tensor_tensor(
                                t2[:], vtmp[:, 144 * j + 48:144 * j + 96],
                                rot9[:, 9 * j + 3 + e:9 * j + 4 + e],
                                t1[:], MULT, ADD)
                            nc.vector.scalar_tensor_tensor(
                                v_nat_v[:, j, :, 33 + 4 * e:37 + 4 * e],
                                vtmp[:, 144 * j + 96:144 * j + 144].rearrange(
                                    "p (h c) -> p h c", h=H),
                                rot9[:, 9 * j + 6 + e:9 * j + 7 + e],
                                t2[:].rearrange("p (h c) -> p h c", h=H), MULT, ADD)

            # ===== interleaved bias stream + flash attention =====
            stage_v2 = stage[:].rearrange("p (j h m) -> p j h m", j=NB, h=H)
            with tc.tile_pool(name="accps", bufs=1, space="PSUM") as accp:
                accT = [accp.tile([128, 512], F32, tag=f"acc{i}", name=f"acc{i}")
                        for i in range(3)]
                with tc.tile_pool(name="biasps", bufs=2, space="PSUM") as bps_pool, \
                     tc.tile_pool(name="lps", bufs=3, space="PSUM") as lpool, \
                     tc.tile_pool(name="esb", bufs=2) as esb:
                    ptj_tiles = []
                    for j in range(2):
                        ptj = pairp.tile([128, M * BL], F8, tag="pair", name="ptj")
                        nc.sync.dma_start(ptj[:], pairT8[:, j])
                        ptj_tiles.append(ptj)

                    def bias_block(j):
                        if j >= 2:
                            ptj = pairp.tile([128, M * BL], F8, tag="pair", name="ptj")
                            if "nodma" not in _ABLATE:
                                nc.sync.dma_start(ptj[:], pairT8[:, j])
                        else:
                            ptj = ptj_tiles[j]
                        if "nobias" in _ABLATE:
                            return
                        for g in range(4):
                            bps = bps_pool.tile([128, 384], F32, tag="bias", name="bps")
                            for mm in range(32):
                                m = 32 * g + mm
                                nc.tensor.matmul(
                                    bps[:, 12 * mm:12 * mm + 12],
                                    ptj[:, BL * m: BL * m + BL], wpb_sb[:, 0:H],
                                    start=True, stop=True)
                            nc.scalar.copy(
                                stage_v2[:, j, :, 32 * g:32 * g + 32],
                                bps[:].rearrange("p (m h) -> p h m", h=H))

                    def flash_block(j):
                        E = esb.tile([128, H * 128], BF16, tag="E", name="E")
                        for hg in range(3):
                            lps = lpool.tile([128, 512], F32, tag="lps", name="lps")
                            for hh in range(4):
                                h = 4 * hg + hh
                                t, u = h // 2, h % 2
                                nc.tensor.matmul(
                                    lps[:, 128 * hh:128 * hh + 128],
                                    ke64[64 * u:64 * u + 64, t * N + j * BL: t * N + j * BL + BL],
                                    qe64[64 * u:64 * u + 64, t * M:(t + 1) * M],
                                    start=(hh == 0), stop=False)
                                nc.tensor.matmul(
                                    lps[:, 128 * hh:128 * hh + 128], idnb,
                                    stage[:, j * 1536 + 128 * h: j * 1536 + 128 * h + 128],
                                    start=False, stop=(hh == 3))
                            nc.scalar.activation(
                                E[:, 512 * hg:512 * (hg + 1)], lps[:], Exp)
                        for h in range(H):
                            nc.tensor.matmul(
                                accT[h // 4][0:45, (h % 4) * 128:(h % 4) * 128 + 128],
                                v_nat[:, 540 * j + 45 * h: 540 * j + 45 * h + 45],
                                E[:, h * 128:(h + 1) * 128],
                                start=(j == 0 and h % 4 == 0),
                                stop=(j == NB - 1 and h % 4 == 3))

                    if "nobias" in _ABLATE:
                        nc.vector.memset(stage[:], 0.0)
                    for j in range(NB):
                        bias_block(j)
                        if j >= 2 and "noflash" not in _ABLATE:
                            flash_block(j - 2)
                    if "noflash" not in _ABLATE:
                        flash_block(NB - 2)
                        flash_block(NB - 1)
                    else:
                        for i in range(3):
                            nc.vector.memset(accT[i][:], 1.0)
                # normalize: cat[ch, m] = acc[ch, m] * (1/den[m]) via outer product
                # den is channel 0; row 0 of cat carries junk (Wcat row is zero)
                with tc.tile_pool(name="obps", bufs=2, space="PSUM") as obp, \
                     tc.tile_pool(name="fin", bufs=2) as fin:
                    for h in range(H):
                        at = accT[h // 4]
                        c0 = (h % 4) * 128
                        rcp = fin.tile([1, 128], F32, tag="rcp", name="rcp")
                        nc.vector.reciprocal(rcp[:], at[0:1, c0:c0 + 128])
                        obc = obp.tile([45, 128], F32, tag="obc", name="obc")
                        nc.tensor.matmul(obc[:], ones44[:, 0:45], rcp[:],
                                         start=True, stop=True)
                        obs = fin.tile([45, 128], F32, tag="obs", name="obs")
                        nc.scalar.copy(obs[:], obc[:])
                        nc.vector.tensor_tensor(
                            cat_sb[64 * (h % 2):64 * (h % 2) + 45, (h // 2) * 128:(h // 2 + 1) * 128],
                            at[0:45, c0:c0 + 128], obs[:], MULT)

            # ============ output projection + residual + LN ============
            with tc.tile_pool(name="fin_sb", bufs=1) as fsb_pool, \
                 tc.tile_pool(name="finps", bufs=1, space="PSUM") as fpool, \
                 tc.tile_pool(name="tps", bufs=2, space="PSUM") as tpool:
                fps = fpool.tile([128, C], F32)
                for b in range(3):
                    for k in range(6):
                        nc.tensor.matmul(
                            fps[:, b * 128:(b + 1) * 128],
                            misc[:, WCAT_O + k * C + b * 128: WCAT_O + k * C + b * 128 + 128],
                            cat_sb[:, k * 128:(k + 1) * 128],
                            start=(k == 0), stop=(k == 5))
                fsb = fsb_pool.tile([128, C], F32)
                for b in range(3):
                    nc.scalar.activation(
                        fsb[:, b * 128:(b + 1) * 128], fps[:, b * 128:(b + 1) * 128],
                        Identity, bias=misc[:, BO_O + b:BO_O + b + 1])
                xres = fsb_pool.tile([128, C], F32)
                for b in range(3):
                    tp = tpool.tile([128, 128], F32)
                    nc.tensor.transpose(tp[:], fsb[:, b * 128:(b + 1) * 128], idn_sb)
                    nc.vector.tensor_tensor(
                        xres[:, b * 128:(b + 1) * 128], tp[:],
                        misc[:, SM_O + b * 128:SM_O + (b + 1) * 128], ADD)
                mu = fsb_pool.tile([128, 1], F32)
                nc.vector.reduce_sum(mu[:], xres[:], axis=mybir.AxisListType.X)
                nc.scalar.mul(mu[:], mu[:], 1.0 / C)
                xc = fsb_pool.tile([128, C], F32)
                nc.vector.tensor_scalar_sub(xc[:], xres[:], mu[:])
                x2 = fsb_pool.tile([128, C], F32)
                var_r = fsb_pool.tile([128, 1], F32)
                nc.scalar.activation(x2[:], xc[:], Square, accum_out=var_r[:])
                epsc = fsb_pool.tile([128, 1], F32)
                nc.vector.memset(epsc[:], EPS)
                stdc = fsb_pool.tile([128, 1], F32)
                nc.scalar.activation(stdc[:], var_r[:], Sqrt, scale=1.0 / C, bias=epsc[:])
                rstd = fsb_pool.tile([128, 1], F32)
                nc.vector.reciprocal(rstd[:], stdc[:])
                xg = fsb_pool.tile([128, C], F32)
                nc.vector.scalar_tensor_tensor(
                    xg[:], xc[:], rstd[:], misc[:, GAM_O:GAM_O + C], MULT, MULT)
                osb = fsb_pool.tile([128, C], F32)
                nc.vector.tensor_tensor(osb[:], xg[:], misc[:, BET_O:BET_O + C], ADD)
                nc.sync.dma_start(OUT[:], osb[:])



def _build_program():
    nc = bass.Bass()
    dp = nc.declare_dram_parameter

    pairT8 = dp("pairT8", [128, NB, M * BL], F8, isOutput=False)  # [c, j, m*128+nw]
    WB = dp("WB", [128, 3 * WCH], BF16, isOutput=False)
    SF = dp("SF", [128, 3 * (N + M)], BF16, isOutput=False)
    MISC = dp("MISC", [128, MISC_W], F32, isOutput=False)
    VB = dp("VB", [128, VBW], F32, isOutput=False)
    KB = [dp(f"KB{g}", [128, KBW], BF16, isOutput=False) for g in range(3)]
    QB = [dp(f"QB{g}", [128, QBW], BF16, isOutput=False) for g in range(3)]
    PB = [dp(f"PB{g}", [128, 6], F32, isOutput=False) for g in range(3)]
    WPB = dp("WPB", [128, H + 128], BF16, isOutput=False)  # wpb | idn(bf16)
    OUT = dp("out", [M, C], F32, isOutput=True)

    for _rep in range(_REPEAT):
        _emit(nc, pairT8, WB, SF, MISC, VB, KB, QB, PB, WPB, OUT)
    return nc


def _bsc(b):
    out = np.zeros((128, H), np.float32)
    for h in range(H):
        u = h % 2
        out[64 * u:64 * u + 32, h] = b[32 * h:32 * h + 32]
    return out


def _qe_init():
    q = np.zeros((128, 6 * 128), np.float32)
    q[44, :] = 1.0
    q[108, :] = 1.0
    return q


def _host_prep(inputs):
    single = np.asarray(inputs["single"], np.float32)
    pair = np.asarray(inputs["pair"], np.float32)
    rot = np.asarray(inputs["rot"], np.float32)
    trans = np.asarray(inputs["trans"], np.float32)
    W = {k: np.asarray(inputs[k], np.float32) for k in
         ["Wq", "bq", "Wk", "bk", "Wv", "bv", "Wpb", "bpb", "Wqp", "bqp",
          "Wkp", "bkp", "Wvp", "bvp", "Wo", "bo", "Wpo", "bpo", "gamma", "beta"]}

    # point weights, rows 32*hh + 4e + p per g-tile of 4 heads (h = 4g + hh)
    def perm_dup(Wp, scale):
        W4 = Wp.reshape(C, H, 4, 3) * scale          # [c, h, p, d]
        out = np.zeros((C, 3, 3, 128), np.float32)   # [c, d, g, row]
        for h in range(H):
            g, hh = h // 4, h % 4
            for e in range(3):
                out[:, :, g, 32 * hh + 4 * e: 32 * hh + 4 * e + 4] = \
                    W4[:, h].transpose(0, 2, 1)      # [c, d, p]
        return out.reshape(C, 3 * 384)

    Wkp3 = perm_dup(W["Wkp"], 1.0)
    Wqp3 = perm_dup(W["Wqp"], SCALE)

    def pb_cols(bp, scale):
        b4 = bp.reshape(H, 4, 3) * scale             # [h, p, d]
        out = np.zeros((3, 128, 3), np.float32)      # [g, row, d]
        for h in range(H):
            g, hh = h // 4, h % 4
            for e in range(3):
                out[g, 32 * hh + 4 * e: 32 * hh + 4 * e + 4, :] = b4[h]
        return out

    bkp3 = pb_cols(W["bkp"], 1.0)
    bqp3 = pb_cols(W["bqp"], SCALE)

    # v combined: scalar [C,384] | points [C,144] in (d, h, p) order
    Wvp_d = W["Wvp"].reshape(C, H, 4, 3).transpose(0, 3, 1, 2).reshape(C, 144)
    Wvc = np.concatenate([W["Wv"], Wvp_d], axis=1)   # [C, 528]

    # WBLOB [128, 3, WCH]
    Wq_s = W["Wq"] * SCALE
    wcat_all = np.concatenate([Wq_s, W["Wk"], Wkp3, Wqp3, Wvc], axis=1)  # [C, WCH]
    WBm = np.ascontiguousarray(wcat_all.reshape(3, 128, WCH).transpose(1, 0, 2)
                               .reshape(128, 3 * WCH)).astype(BF)

    # KBLOB per g: rb[d] 3*1024 | tb 1024 | sel 12; rows 32hh + 4e + p
    rb_k = np.zeros((3, 128, KBW), np.float32)
    for g in range(3):
        for hh in range(4):
            h = 4 * g + hh
            for e in range(3):
                r = 32 * hh + 4 * e
                for d in range(3):
                    rb_k[g, r:r + 4, KRB_O + d * N:KRB_O + (d + 1) * N] = rot[0, :, d, e]
                rb_k[g, r:r + 4, KTB_O:KTB_O + N] = trans[0, :, e]
                rb_k[g, r:r + 4, KSEL_O + h] = -0.5 * SCALE

    # Wcat for output projection; cat rows 64u + 1 + i (row 64u is junk)
    Wcat = np.zeros((6, 128, C), np.float32)
    Wpo4 = W["Wpo"].reshape(H, 4, 3, C)
    for h in range(H):
        blk, ro = h // 2, 64 * (h % 2) + 1
        Wcat[blk, ro:ro + 32] = W["Wo"][32 * h:32 * h + 32]
        for e in range(3):
            for p in range(4):
                Wcat[blk, ro + 32 + 4 * e + p] = Wpo4[h, p, e]

    bo_col = W["bo"] + W["bpo"] + W["bv"] @ W["Wo"]  # [C]

    # v-point offsets: tbv[n, 48e+4h+p] = sum_d bvp[h,p,d] rot[n,d,e] + trans[n,e]
    bvp3 = W["bvp"].reshape(H, 4, 3)
    tbv = np.einsum('hpd,nde->nehp', bvp3, rot[0]) + trans[0][:, :, None, None]
    VBm = np.zeros((128, VBW), np.float32)
    VBm[:, VROT_O:VROT_O + 72] = rot[0].reshape(NB, 128, 9).transpose(1, 0, 2).reshape(128, 72)
    VBm[:, VTBV_O:] = tbv.reshape(NB, 128, 144).transpose(1, 0, 2).reshape(128, 1152)

    shared = {
        "WB": WBm,
        "WPB": np.concatenate(
            [W["Wpb"], np.eye(128, dtype=np.float32)], axis=1).astype(BF),
        "VB": VBm,
    }
    for g in range(3):
        shared[f"KB{g}"] = rb_k[g].astype(BF)
        shared[f"PB{g}"] = np.ascontiguousarray(
            np.concatenate([bkp3[g], bqp3[g]], axis=1))

    in_maps = []
    for c in range(NCORES):
        m0 = c * M
        im = dict(shared)
        # pair [c, n, m] -> [c, j, m, nw]
        pT = pair[0, m0:m0 + M].transpose(2, 1, 0).reshape(128, NB, BL, M)
        pT = pT.transpose(0, 1, 3, 2).reshape(128, NB, M * BL)
        im["pairT8"] = np.ascontiguousarray(pT).astype(E4M3)

        # SF: [128, 3, N + M]: single^T full | single^T m-slice
        sfm = np.concatenate([single[0].T, single[0, m0:m0 + M].T], axis=1)  # [C, N+M]
        im["SF"] = np.ascontiguousarray(
            sfm.reshape(3, 128, N + M).transpose(1, 0, 2)
            .reshape(128, 3 * (N + M))).astype(BF)

        # QBLOB per g: rb 3*128 | tb 128, m-slice rotations
        rb_q = np.zeros((3, 128, QBW), np.float32)
        for g in range(3):
            for hh in range(4):
                for e in range(3):
                    r = 32 * hh + 4 * e
                    for d in range(3):
                        rb_q[g, r:r + 4, QRB_O + d * M:QRB_O + (d + 1) * M] = \
                            rot[0, m0:m0 + M, d, e]
                    rb_q[g, r:r + 4, QTB_O:QTB_O + M] = trans[0, m0:m0 + M, e] * SCALE
        for g in range(3):
            im[f"QB{g}"] = rb_q[g].astype(BF)

        miscm = np.zeros((128, MISC_W), np.float32)
        miscm[:, IDN_O:IDN_O + 128] = np.eye(128, dtype=np.float32)
        miscm[:, QEI_O:QEI_O + 768] = _qe_init()
        miscm[:, BSCK_O:BSCK_O + H] = _bsc(W["bk"])
        miscm[:, BSCQ_O:BSCQ_O + H] = _bsc(W["bq"] * SCALE)
        miscm[:, GAM_O:GAM_O + C] = np.broadcast_to(W["gamma"], (128, C))
        miscm[:, BET_O:BET_O + C] = np.broadcast_to(W["beta"], (128, C))
        miscm[:, SM_O:SM_O + C] = single[0, m0:m0 + M]
        miscm[:, WCAT_O:WCAT_O + 6 * C] = Wcat.transpose(1, 0, 2).reshape(128, 6 * C)
        miscm[:, BO_O:BO_O + 3] = bo_col.reshape(3, 128).T
        im["MISC"] = miscm
        in_maps.append(im)
    return in_maps


_NC_CACHE = {}


def get_nc():
    if "nc" not in _NC_CACHE:
        _NC_CACHE["nc"] = _build_program()
    return _NC_CACHE["nc"]


def kernel(**inputs) -> np.ndarray:
    mask = np.asarray(inputs["mask"])
    assert mask.all(), "kernel assumes all-ones mask"
    nc = get_nc()
    in_maps = _host_prep(inputs)
    res = run_bass_kernel_spmd(nc, in_maps, core_ids=list(range(NCORES)))
    out = np.concatenate([np.asarray(res.results[c]["out"]) for c in range(NCORES)], axis=0)
    return out.reshape(1, N, C).astype(np.float32)



# revision 51
# speedup vs baseline: 1.8355x; 1.8355x over previous
"""IPA (invariant point attention) Trainium2 kernel, 8-core SPMD, query-row sharded.

v3m: v3 flash pipeline with all inputs merged into 3 dram params.
  - launch overhead is ~45-60us PER INPUT PARAM through the axon/PJRT
    custom-call path (measured: 14 params ~645us extra vs 3 params ~0);
    all weights/constants ship as one bf16 blob + one f32 blob + pair fp8,
    and the unused partition_id input is disabled
  - v3 body: flash-style pipeline with transposed logits, on-chip rotation.
  - pair host-transposed to [c, block, m, n_within] fp8(e4m3); Wpb bf16
  - bias matmuls produce [n_block, 12h] tiles (n on partitions), staged per
    block in bf16 (h-major); logits computed transposed L^T[n, m] via one
    matmul + an identity-matmul that accumulates the staged bias into the
    same PSUM group; exp gives E^T directly; attn@v accumulates [45ch, m]
    in PSUM across blocks with a ones-channel (channel 0) giving softmax
    denominators; normalized at the end via a reciprocal outer-product
  - point projections emit rows padded to 32-row spacing per head and
    duplicated per rotation coordinate (h,e,p), so rotation = row-wise
    multiply-add against per-row broadcast tensors, and rotated features
    land in ke64/qe64 via 32-aligned on-chip DVE copies (no DRAM roundtrip)
  - v projections emitted token-major directly; v-point rotation via fused
    scalar_tensor_tensor with per-token rot scalars on partitions
  - q2 term dropped (softmax-invariant); pair-bias bias bpb dropped
    (softmax-invariant); bv folded into bo_col; scale folded into weights
"""
import os
import sys
sys.path.insert(0, '/opt/trn_rl_repo')

import numpy as np
import ml_dtypes

_ABLATE = set(os.environ.get("IPA_ABLATE", "").split(","))
_REPEAT = int(os.environ.get("IPA_REPEAT", "1"))
_MERGE = os.environ.get("IPA_MERGE", "1") == "1"
_DMA0 = os.environ.get("IPA_DMA0", "0") == "1"

import concourse.bass as bass
import concourse.mybir as mybir
from concourse.tile import TileContext
from concourse.vector_clock import ScopedClock
from concourse.bass_utils import run_bass_kernel_spmd

F32 = mybir.dt.float32
F16 = mybir.dt.float16
BF16 = mybir.dt.bfloat16
F8 = mybir.dt.float8e4
BF = ml_dtypes.bfloat16
E4M3 = ml_dtypes.float8_e4m3

N = 1024
M = 128
C = 384
H = 12
NB = 8          # n blocks
BL = 128        # block length
NCORES = 8
SCALE = (C // H) ** -0.5
EPS = 1e-5
Exp = mybir.ActivationFunctionType.Exp
Identity = mybir.ActivationFunctionType.Identity
Sqrt = mybir.ActivationFunctionType.Sqrt
Square = mybir.ActivationFunctionType.Square
ADD = mybir.AluOpType.add
MULT = mybir.AluOpType.mult

# WBLOB column offsets (per 128-row c-chunk)
WQ_O, WK_O, WKP_O, WQP_O, WVC_O = 0, 384, 768, 1920, 3072
WCH = 3600
# MISC column offsets
IDN_O, QEI_O, BSCK_O, BSCQ_O, GAM_O, BET_O, SM_O, WCAT_O, BO_O = \
    0, 128, 896, 908, 920, 1304, 1688, 2072, 4376
MISC_W = 4379
# KBLOB (per g, 128 rows): rb 3*1024 | tb 1024 | sel 12
KRB_O, KTB_O, KSEL_O = 0, 3072, 4096
KBW = 4108
# QBLOB (per g, 128 rows): rb 3*128 | tb 128
QRB_O, QTB_O = 0, 384
QBW = 512
# VBLOB: rot9 (8*9) | tbv (8*144)
VROT_O, VTBV_O = 0, 72
VBW = 72 + 1152

_MAXW = int(os.environ.get("IPA_MAXW", "1"))


def _patched_drain_and_barrier(self, tick_clock, wait_clock):
    # walrus rejects >2 sync waits on one Drain; split tail waits across nops
    nc = self.nc
    probe = nc.sync.nop()
    wait_clock.add_sem_waits(probe.ins, ScopedClock({None: tick_clock.global_clock}))
    waits = list(probe.ins.sync_info.on_wait or [])
    probe.ins.sync_info.on_wait = waits[:_MAXW]
    rest = waits[_MAXW:]
    while rest:
        n2 = nc.sync.nop()
        n2.ins.sync_info = mybir.SyncInfo(on_wait=rest[:_MAXW], on_update=[])
        rest = rest[_MAXW:]
    nc.sync.drain()
    nc.all_engine_barrier()
    assert self.sems is not None
    popped = nc._tile_sem_poison_stack.pop()
    assert popped is self._sem_poison
    nc.clear_and_free_semaphores(list(self.sems.allocated().values()))
    nc.all_engine_barrier()


TileContext._drain_and_barrier = _patched_drain_and_barrier

_orig_lower_ordered = TileContext._lower_ordered_insts


def _split_waits_then_lower(self, ordered):
    # HW instructions encode a limited number of sync waits; hoist excess
    # onto NoOps inserted immediately before, on the same engine.
    nc = self.nc
    for bb in list(ordered.keys()):
        insts = ordered[bb]
        new = []
        for inst in insts:
            si = getattr(inst, "sync_info", None)
            if si is not None and si.on_wait and len(si.on_wait) > _MAXW:
                waits = list(si.on_wait)
                while len(waits) > _MAXW:
                    chunk, waits = waits[:_MAXW], waits[_MAXW:]
                    nop = mybir.InstNoOp(
                        name=nc.get_next_instruction_name(),
                        engine=inst.engine, ins=[], outs=[], bass_nofuse=True,
                        sync_info=mybir.SyncInfo(on_wait=chunk, on_update=[]))
                    new.append(nop)
                si.on_wait = waits
            new.append(inst)
        ordered[bb] = new
    return _orig_lower_ordered(self, ordered)


TileContext._lower_ordered_insts = _split_waits_then_lower


def _emit(nc, pairT8, WB, SF, MISC, VB, KB, QB, PB, WPB, OUT):
    with TileContext(nc) as tc:
        with tc.tile_pool(name="persist", bufs=1) as pp, \
             tc.tile_pool(name="pair", bufs=2) as pairp, \
             tc.tile_pool(name="dstage", bufs=1, space="DRAM") as dstp:
            qe64 = pp.tile([128, 6 * M], BF16)
            ke64 = pp.tile([128, 6 * N], BF16)
            v_nat = pp.tile([128, NB * 540], BF16)
            stage = pp.tile([128, NB * 1536], BF16)
            wpb_sb = pp.tile([128, H + 128], BF16)
            misc = pp.tile([128, MISC_W], F32)
            vb_sb = pp.tile([128, VBW], F32)
            ones44 = pp.tile([1, 64], F32)
            cat_sb = pp.tile([128, 6 * 128], F32)
            k2sb = pp.tile([H, N], F32)
            k2_dram = dstp.tile([H, N], F32)

            nc.sync.dma_start(wpb_sb[:], WPB[:])
            nc.sync.dma_start(misc[:], MISC[:])
            nc.sync.dma_start(vb_sb[:], VB[:])
            nc.vector.memset(ke64[:], 0.0)
            nc.vector.memset(ones44[:], 1.0)
            nc.vector.memset(cat_sb[:], 0.0)
            nc.vector.tensor_copy(qe64[:], misc[:, QEI_O:QEI_O + 768])
            v_nat_v = v_nat[:].rearrange("p (j h c) -> p j h c", j=NB, h=H)
            nc.vector.memset(v_nat_v[:, :, :, 0:1], 1.0)

            idn_sb = misc[:, IDN_O:IDN_O + 128]
            idnb = wpb_sb[:, H:H + 128]

            # ====== phase A ======
            with tc.tile_pool(name="wload", bufs=1) as wl, \
                 tc.tile_pool(name="rot", bufs=2) as rp:
                wb = wl.tile([128, 3 * WCH], BF16)
                nc.sync.dma_start(wb[:], WB[:])
                sf = wl.tile([128, 3 * (N + M)], BF16)
                nc.sync.dma_start(sf[:], SF[:])
                kb = []
                qb = []
                pb = []
                for g in range(3):
                    t = wl.tile([128, KBW], BF16, tag=f"kb{g}", name=f"kb{g}")
                    nc.sync.dma_start(t[:], KB[g][:])
                    kb.append(t)
                    t = wl.tile([128, QBW], BF16, tag=f"qb{g}", name=f"qb{g}")
                    nc.sync.dma_start(t[:], QB[g][:])
                    qb.append(t)
                    t = wl.tile([128, 6], F32, tag=f"pb{g}", name=f"pb{g}")
                    nc.sync.dma_start(t[:], PB[g][:])
                    pb.append(t)
                kpT3 = [wl.tile([128, 3 * N], BF16, tag=f"kpT{g}", name=f"kpT{g}")
                        for g in range(3)]
                qpT3 = [wl.tile([128, 3 * M], BF16, tag=f"qpT{g}", name=f"qpT{g}")
                        for g in range(3)]
                vtmp = wl.tile([128, NB * 144], F32)

                def sfT(ci, o, w):
                    return sf[:, ci * (N + M) + o: ci * (N + M) + o + w]

                def smT(ci, o, w):
                    return sf[:, ci * (N + M) + N + o: ci * (N + M) + N + o + w]

                with tc.tile_pool(name="p1ps", bufs=2, space="PSUM") as pps, \
                     tc.tile_pool(name="vps", bufs=2, space="PSUM") as vps, \
                     tc.tile_pool(name="k2ps", bufs=1, space="PSUM") as k2pool:
                    # ---- scalar q/k projections (feature-major into ke64/qe64) ----
                    def grp_scal(wo, dste, bsc_o, mov, width, co, nb):
                        o = nb * 512
                        w = min(512, width - o)
                        ps = pps.tile([128, 512], F32, tag="proj", name="ps")
                        for ci in range(3):
                            nc.tensor.matmul(
                                ps[:, 0:w],
                                wb[:, ci * WCH + wo + co * 128: ci * WCH + wo + co * 128 + 128],
                                mov(ci, o, w),
                                start=(ci == 0), stop=(ci == 2))
                        for hh in range(4):
                            h = 4 * co + hh
                            t, u = h // 2, h % 2
                            nc.scalar.activation(
                                dste[64 * u:64 * u + 32, t * width + o: t * width + o + w],
                                ps[32 * hh:32 * hh + 32, 0:w], Identity,
                                bias=misc[64 * u:64 * u + 32, bsc_o + h:bsc_o + h + 1])

                    for co in range(3):
                        for nb in range(2):
                            grp_scal(WK_O, ke64, BSCK_O, sfT, N, co, nb)
                    for co in range(3):
                        grp_scal(WQ_O, qe64, BSCQ_O, smT, M, co, 0)

                    # ---- point projections, rows 32*hh + 4e + p per g of 4 heads ----
                    def grp_pt3(wo, dst, pbt, dcol, mov, width, g, d, nb):
                        o = nb * 512
                        w = min(512, width - o)
                        ps = pps.tile([128, 512], F32, tag="proj", name="ps")
                        for ci in range(3):
                            nc.tensor.matmul(
                                ps[:, 0:w],
                                wb[:, ci * WCH + wo + d * 384 + 128 * g:
                                   ci * WCH + wo + d * 384 + 128 * g + 128],
                                mov(ci, o, w),
                                start=(ci == 0), stop=(ci == 2))
                        nc.scalar.activation(
                            dst[:, d * width + o: d * width + o + w],
                            ps[:, 0:w], Identity, bias=pbt[:, dcol + d:dcol + d + 1])

                    for g in range(3):
                        for d in range(3):
                            for nb in range(2):
                                grp_pt3(WKP_O, kpT3[g], pb[g], 0, sfT, N, g, d, nb)
                    for g in range(3):
                        for d in range(3):
                            grp_pt3(WQP_O, qpT3[g], pb[g], 3, smT, M, g, d, 0)

                    # ---- v projections: token-major direct ----
                    for j in range(NB):
                        vs_ps = vps.tile([128, 384], F32, tag="vs", name="vs_ps")
                        vp_ps = vps.tile([128, 144], F32, tag="vp", name="vp_ps")
                        for ci in range(3):
                            nc.tensor.matmul(
                                vs_ps[:],
                                sfT(ci, j * BL, BL),
                                wb[:, ci * WCH + WVC_O: ci * WCH + WVC_O + 384],
                                start=(ci == 0), stop=(ci == 2))
                            nc.tensor.matmul(
                                vp_ps[:],
                                sfT(ci, j * BL, BL),
                                wb[:, ci * WCH + WVC_O + 384: ci * WCH + WVC_O + 528],
                                start=(ci == 0), stop=(ci == 2))
                        nc.scalar.copy(
                            v_nat_v[:, j, :, 1:33],
                            vs_ps[:].rearrange("p (h c) -> p h c", h=H))
                        nc.scalar.copy(vtmp[:, 144 * j:144 * (j + 1)], vp_ps[:])

                    # ---- k rotation on-chip: rc = sum_d kp_d * rb_d + tb ----
                    RCH = 512
                    k2t = [k2pool.tile([H, 512], F32, tag=f"k2_{half}", name=f"k2t{half}")
                           for half in range(2)]

                    def rot_chain(src, blob, rbo, tbo, width, o, w, tag):
                        rc = rp.tile([128, RCH], BF16, tag=f"rc{tag}", name="rc")
                        t2 = rp.tile([128, RCH], BF16, tag=f"rt{tag}", name="t2")
                        nc.vector.tensor_tensor(
                            rc[:, 0:w], src[:, o:o + w],
                            blob[:, rbo + o:rbo + o + w], MULT)
                        nc.vector.tensor_tensor(
                            t2[:, 0:w], src[:, width + o:width + o + w],
                            blob[:, rbo + width + o:rbo + width + o + w], MULT)
                        nc.vector.tensor_tensor(rc[:, 0:w], rc[:, 0:w], t2[:, 0:w], ADD)
                        nc.vector.tensor_tensor(
                            t2[:, 0:w], src[:, 2 * width + o:2 * width + o + w],
                            blob[:, rbo + 2 * width + o:rbo + 2 * width + o + w], MULT)
                        nc.vector.tensor_tensor(rc[:, 0:w], rc[:, 0:w], t2[:, 0:w], ADD)
                        nc.vector.tensor_tensor(
                            rc[:, 0:w], rc[:, 0:w],
                            blob[:, tbo + o:tbo + o + w], ADD)
                        return rc

                    for g in range(3):
                        for ci_ in range(2):
                            o = ci_ * RCH
                            rc = rot_chain(kpT3[g], kb[g], KRB_O, KTB_O, N, o, RCH, "k")
                            sq = rp.tile([128, RCH], BF16, tag="sq", name="sq")
                            nc.vector.tensor_tensor(sq[:], rc[:], rc[:], MULT)
                            nc.tensor.matmul(
                                k2t[ci_][:], kb[g][:, KSEL_O:KSEL_O + 12], sq[:],
                                start=(g == 0), stop=(g == 2))
                            for hh in range(4):
                                h = 4 * g + hh
                                t, u = h // 2, h % 2
                                nc.vector.tensor_copy(
                                    ke64[64 * u + 32:64 * u + 44, t * N + o: t * N + o + RCH],
                                    rc[32 * hh:32 * hh + 12, :])
                        qrc = rot_chain(qpT3[g], qb[g], QRB_O, QTB_O, M, 0, M, "q")
                        for hh in range(4):
                            h = 4 * g + hh
                            t, u = h // 2, h % 2
                            nc.vector.tensor_copy(
                                qe64[64 * u + 32:64 * u + 44, t * M:(t + 1) * M],
                                qrc[32 * hh:32 * hh + 12, 0:M])

                    # k2 rows into ke64 (partition scatter via DRAM roundtrip)
                    for half in range(2):
                        nc.vector.tensor_copy(
                            k2sb[:, half * 512:(half + 1) * 512], k2t[half][:])
                    nc.gpsimd.dma_start(k2_dram[:], k2sb[:])
                    for u in range(2):
                        dst = ke64[64 * u + 44: 64 * u + 45, :].rearrange(
                            "one (t n) -> one t n", t=6)
                        src = k2_dram[:].rearrange("(t u2) n -> u2 t n", u2=2)[u:u + 1]
                        nc.gpsimd.dma_start(dst, src)

                    # ---- v-point rotation via per-token scalars ----
                    rot9 = vb_sb[:, VROT_O:VROT_O + 72]
                    tbv = vb_sb[:, VTBV_O:VTBV_O + 1152]
                    for j in range(NB):
                        for e in range(3):
                            t1 = rp.tile([128, 48], F32, tag="vst1", name="t1")
                            t2 = rp.tile([128, 48], F32, tag="vst2", name="t2")
                            nc.vector.scalar_tensor_tensor(
                                t1[:], vtmp[:, 144 * j:144 * j + 48],
                                rot9[:, 9 * j + e:9 * j + e + 1],
                                tbv[:, 144 * j + 48 * e: 144 * j + 48 * e + 48], MULT, ADD)
                            nc.vector.scalar_tensor_tensor(
                                t2[:], vtmp[:, 144 * j + 48:144 * j + 96],
                                rot9[:, 9 * j + 3 + e:9 * j + 4 + e],
                                t1[:], MULT, ADD)
                            nc.vector.scalar_tensor_tensor(
                                v_nat_v[:, j, :, 33 + 4 * e:37 + 4 * e],
                                vtmp[:, 144 * j + 96:144 * j + 144].rearrange(
                                    "p (h c) -> p h c", h=H),
                                rot9[:, 9 * j + 6 + e:9 * j + 7 + e],
                                t2[:].rearrange("p (h c) -> p h c", h=H), MULT, ADD)

            # ===== interleaved bias stream + flash attention =====
            stage_v2 = stage[:].rearrange("p (j h m) -> p j h m", j=NB, h=H)
            with tc.tile_pool(name="accps", bufs=1, space="PSUM") as accp:
                accT = [accp.tile([128, 512], F32, tag=f"acc{i}", name=f"acc{i}")
                        for i in range(3)]
                with tc.tile_pool(name="biasps", bufs=2, space="PSUM") as bps_pool, \
                     tc.tile_pool(name="lps", bufs=3, space="PSUM") as lpool, \
                     tc.tile_pool(name="esb", bufs=2) as esb:
                    ptj_tiles = []
                    for j in range(2):
                        ptj = pairp.tile([128, M * BL], F8, tag="pair", name="ptj")
                        nc.sync.dma_start(ptj[:], pairT8[:, 0 if _DMA0 else j])
                        ptj_tiles.append(ptj)

                    def bias_block(j):
                        if j >= 2:
                            ptj = pairp.tile([128, M * BL], F8, tag="pair", name="ptj")
                            if "nodma" not in _ABLATE:
                                nc.sync.dma_start(ptj[:], pairT8[:, 0 if _DMA0 else j])
                        else:
                            ptj = ptj_tiles[j]
                        if "nobias" in _ABLATE:
                            return
                        for g in range(4):
                            bps = bps_pool.tile([128, 384], F32, tag="bias", name="bps")
                            for mm in range(32):
                                m = 32 * g + mm
                                nc.tensor.matmul(
                                    bps[:, 12 * mm:12 * mm + 12],
                                    ptj[:, BL * m: BL * m + BL], wpb_sb[:, 0:H],
                                    start=True, stop=True)
                            nc.scalar.copy(
                                stage_v2[:, j, :, 32 * g:32 * g + 32],
                                bps[:].rearrange("p (m h) -> p h m", h=H))

                    def flash_block(j):
                        E = esb.tile([128, H * 128], BF16, tag="E", name="E")
                        for hg in range(3):
                            lps = lpool.tile([128, 512], F32, tag="lps", name="lps")
                            for hh in range(4):
                                h = 4 * hg + hh
                                t, u = h // 2, h % 2
                                nc.tensor.matmul(
                                    lps[:, 128 * hh:128 * hh + 128],
                                    ke64[64 * u:64 * u + 64, t * N + j * BL: t * N + j * BL + BL],
                                    qe64[64 * u:64 * u + 64, t * M:(t + 1) * M],
                                    start=(hh == 0), stop=False)
                                nc.tensor.matmul(
                                    lps[:, 128 * hh:128 * hh + 128], idnb,
                                    stage[:, j * 1536 + 128 * h: j * 1536 + 128 * h + 128],
                                    start=False, stop=(hh == 3))
                            nc.scalar.activation(
                                E[:, 512 * hg:512 * (hg + 1)], lps[:], Exp)
                        for h in range(H):
                            nc.tensor.matmul(
                                accT[h // 4][0:45, (h % 4) * 128:(h % 4) * 128 + 128],
                                v_nat[:, 540 * j + 45 * h: 540 * j + 45 * h + 45],
                                E[:, h * 128:(h + 1) * 128],
                                start=(j == 0 and h % 4 == 0),
                                stop=(j == NB - 1 and h % 4 == 3))

                    if "nobias" in _ABLATE:
                        nc.vector.memset(stage[:], 0.0)
                    for j in range(NB):
                        bias_block(j)
                        if j >= 2 and "noflash" not in _ABLATE:
                            flash_block(j - 2)
                    if "noflash" not in _ABLATE:
                        flash_block(NB - 2)
                        flash_block(NB - 1)
                    else:
                        for i in range(3):
                            nc.vector.memset(accT[i][:], 1.0)
                # normalize: cat[ch, m] = acc[ch, m] * (1/den[m]) via outer product
                # den is channel 0; row 0 of cat carries junk (Wcat row is zero)
                with tc.tile_pool(name="obps", bufs=2, space="PSUM") as obp, \
                     tc.tile_pool(name="fin", bufs=2) as fin:
                    for h in range(H):
                        at = accT[h // 4]
                        c0 = (h % 4) * 128
                        rcp = fin.tile([1, 128], F32, tag="rcp", name="rcp")
                        nc.vector.reciprocal(rcp[:], at[0:1, c0:c0 + 128])
                        obc = obp.tile([45, 128], F32, tag="obc", name="obc")
                        nc.tensor.matmul(obc[:], ones44[:, 0:45], rcp[:],
                                         start=True, stop=True)
                        obs = fin.tile([45, 128], F32, tag="obs", name="obs")
                        nc.scalar.copy(obs[:], obc[:])
                        nc.vector.tensor_tensor(
                            cat_sb[64 * (h % 2):64 * (h % 2) + 45, (h // 2) * 128:(h // 2 + 1) * 128],
                            at[0:45, c0:c0 + 128], obs[:], MULT)

            # ============ output projection + residual + LN ============
            with tc.tile_pool(name="fin_sb", bufs=1) as fsb_pool, \
                 tc.tile_pool(name="finps", bufs=1, space="PSUM") as fpool, \
                 tc.tile_pool(name="tps", bufs=2, space="PSUM") as tpool:
                fps = fpool.tile([128, C], F32)
                for b in range(3):
                    for k in range(6):
                        nc.tensor.matmul(
                            fps[:, b * 128:(b + 1) * 128],
                            misc[:, WCAT_O + k * C + b * 128: WCAT_O + k * C + b * 128 + 128],
                            cat_sb[:, k * 128:(k + 1) * 128],
                            start=(k == 0), stop=(k == 5))
                fsb = fsb_pool.tile([128, C], F32)
                for b in range(3):
                    nc.scalar.activation(
                        fsb[:, b * 128:(b + 1) * 128], fps[:, b * 128:(b + 1) * 128],
                        Identity, bias=misc[:, BO_O + b:BO_O + b + 1])
                xres = fsb_pool.tile([128, C], F32)
                for b in range(3):
                    tp = tpool.tile([128, 128], F32)
                    nc.tensor.transpose(tp[:], fsb[:, b * 128:(b + 1) * 128], idn_sb)
                    nc.vector.tensor_tensor(
                        xres[:, b * 128:(b + 1) * 128], tp[:],
                        misc[:, SM_O + b * 128:SM_O + (b + 1) * 128], ADD)
                mu = fsb_pool.tile([128, 1], F32)
                nc.vector.reduce_sum(mu[:], xres[:], axis=mybir.AxisListType.X)
                nc.scalar.mul(mu[:], mu[:], 1.0 / C)
                xc = fsb_pool.tile([128, C], F32)
                nc.vector.tensor_scalar_sub(xc[:], xres[:], mu[:])
                x2 = fsb_pool.tile([128, C], F32)
                var_r = fsb_pool.tile([128, 1], F32)
                nc.scalar.activation(x2[:], xc[:], Square, accum_out=var_r[:])
                epsc = fsb_pool.tile([128, 1], F32)
                nc.vector.memset(epsc[:], EPS)
                stdc = fsb_pool.tile([128, 1], F32)
                nc.scalar.activation(stdc[:], var_r[:], Sqrt, scale=1.0 / C, bias=epsc[:])
                rstd = fsb_pool.tile([128, 1], F32)
                nc.vector.reciprocal(rstd[:], stdc[:])
                xg = fsb_pool.tile([128, C], F32)
                nc.vector.scalar_tensor_tensor(
                    xg[:], xc[:], rstd[:], misc[:, GAM_O:GAM_O + C], MULT, MULT)
                osb = fsb_pool.tile([128, C], F32)
                nc.vector.tensor_tensor(osb[:], xg[:], misc[:, BET_O:BET_O + C], ADD)
                nc.sync.dma_start(OUT[:], osb[:])



# merged-blob column offsets (bf16 blob / f32 blob)
_BF_W = 3 * WCH + 3 * (N + M) + 3 * KBW + 3 * QBW + (H + 128)
_F32_W = MISC_W + VBW + 3 * 6


def _bf_offsets():
    o = {}
    c = 0
    o["WB"] = c; c += 3 * WCH
    o["SF"] = c; c += 3 * (N + M)
    for g in range(3):
        o[f"KB{g}"] = c; c += KBW
    for g in range(3):
        o[f"QB{g}"] = c; c += QBW
    o["WPB"] = c; c += H + 128
    assert c == _BF_W
    return o


def _f32_offsets():
    o = {}
    c = 0
    o["MISC"] = c; c += MISC_W
    o["VB"] = c; c += VBW
    for g in range(3):
        o[f"PB{g}"] = c; c += 6
    assert c == _F32_W
    return o


def _build_program():
    nc = bass.Bass(enable_partition_id=False)
    dp = nc.declare_dram_parameter

    pairT8 = dp("pairT8", [128, NB, M * BL], F8, isOutput=False)  # [c, j, m*128+nw]
    if _MERGE:
        BFB = dp("BFB", [128, _BF_W], BF16, isOutput=False)
        F32B = dp("F32B", [128, _F32_W], F32, isOutput=False)
        ob, of = _bf_offsets(), _f32_offsets()

        def bsl(k, w):
            return BFB[:, ob[k]:ob[k] + w]

        def fsl(k, w):
            return F32B[:, of[k]:of[k] + w]

        WB = bsl("WB", 3 * WCH)
        SF = bsl("SF", 3 * (N + M))
        KB = [bsl(f"KB{g}", KBW) for g in range(3)]
        QB = [bsl(f"QB{g}", QBW) for g in range(3)]
        WPB = bsl("WPB", H + 128)
        MISC = fsl("MISC", MISC_W)
        VB = fsl("VB", VBW)
        PB = [fsl(f"PB{g}", 6) for g in range(3)]
    else:
        WB = dp("WB", [128, 3 * WCH], BF16, isOutput=False)
        SF = dp("SF", [128, 3 * (N + M)], BF16, isOutput=False)
        MISC = dp("MISC", [128, MISC_W], F32, isOutput=False)
        VB = dp("VB", [128, VBW], F32, isOutput=False)
        KB = [dp(f"KB{g}", [128, KBW], BF16, isOutput=False) for g in range(3)]
        QB = [dp(f"QB{g}", [128, QBW], BF16, isOutput=False) for g in range(3)]
        PB = [dp(f"PB{g}", [128, 6], F32, isOutput=False) for g in range(3)]
        WPB = dp("WPB", [128, H + 128], BF16, isOutput=False)  # wpb | idn(bf16)
    OUT = dp("out", [M, C], F32, isOutput=True)

    for _rep in range(_REPEAT):
        _emit(nc, pairT8, WB, SF, MISC, VB, KB, QB, PB, WPB, OUT)
    return nc


def _bsc(b):
    out = np.zeros((128, H), np.float32)
    for h in range(H):
        u = h % 2
        out[64 * u:64 * u + 32, h] = b[32 * h:32 * h + 32]
    return out


def _qe_init():
    q = np.zeros((128, 6 * 128), np.float32)
    q[44, :] = 1.0
    q[108, :] = 1.0
    return q


def _host_prep(inputs):
    single = np.asarray(inputs["single"], np.float32)
    pair = np.asarray(inputs["pair"], np.float32)
    rot = np.asarray(inputs["rot"], np.float32)
    trans = np.asarray(inputs["trans"], np.float32)
    W = {k: np.asarray(inputs[k], np.float32) for k in
         ["Wq", "bq", "Wk", "bk", "Wv", "bv", "Wpb", "bpb", "Wqp", "bqp",
          "Wkp", "bkp", "Wvp", "bvp", "Wo", "bo", "Wpo", "bpo", "gamma", "beta"]}

    # point weights, rows 32*hh + 4e + p per g-tile of 4 heads (h = 4g + hh)
    def perm_dup(Wp, scale):
        W4 = Wp.reshape(C, H, 4, 3) * scale          # [c, h, p, d]
        out = np.zeros((C, 3, 3, 128), np.float32)   # [c, d, g, row]
        for h in range(H):
            g, hh = h // 4, h % 4
            for e in range(3):
                out[:, :, g, 32 * hh + 4 * e: 32 * hh + 4 * e + 4] = \
                    W4[:, h].transpose(0, 2, 1)      # [c, d, p]
        return out.reshape(C, 3 * 384)

    Wkp3 = perm_dup(W["Wkp"], 1.0)
    Wqp3 = perm_dup(W["Wqp"], SCALE)

    def pb_cols(bp, scale):
        b4 = bp.reshape(H, 4, 3) * scale             # [h, p, d]
        out = np.zeros((3, 128, 3), np.float32)      # [g, row, d]
        for h in range(H):
            g, hh = h // 4, h % 4
            for e in range(3):
                out[g, 32 * hh + 4 * e: 32 * hh + 4 * e + 4, :] = b4[h]
        return out

    bkp3 = pb_cols(W["bkp"], 1.0)
    bqp3 = pb_cols(W["bqp"], SCALE)

    # v combined: scalar [C,384] | points [C,144] in (d, h, p) order
    Wvp_d = W["Wvp"].reshape(C, H, 4, 3).transpose(0, 3, 1, 2).reshape(C, 144)
    Wvc = np.concatenate([W["Wv"], Wvp_d], axis=1)   # [C, 528]

    # WBLOB [128, 3, WCH]
    Wq_s = W["Wq"] * SCALE
    wcat_all = np.concatenate([Wq_s, W["Wk"], Wkp3, Wqp3, Wvc], axis=1)  # [C, WCH]
    WBm = np.ascontiguousarray(wcat_all.reshape(3, 128, WCH).transpose(1, 0, 2)
                               .reshape(128, 3 * WCH)).astype(BF)

    # KBLOB per g: rb[d] 3*1024 | tb 1024 | sel 12; rows 32hh + 4e + p
    rb_k = np.zeros((3, 128, KBW), np.float32)
    for g in range(3):
        for hh in range(4):
            h = 4 * g + hh
            for e in range(3):
                r = 32 * hh + 4 * e
                for d in range(3):
                    rb_k[g, r:r + 4, KRB_O + d * N:KRB_O + (d + 1) * N] = rot[0, :, d, e]
                rb_k[g, r:r + 4, KTB_O:KTB_O + N] = trans[0, :, e]
                rb_k[g, r:r + 4, KSEL_O + h] = -0.5 * SCALE

    # Wcat for output projection; cat rows 64u + 1 + i (row 64u is junk)
    Wcat = np.zeros((6, 128, C), np.float32)
    Wpo4 = W["Wpo"].reshape(H, 4, 3, C)
    for h in range(H):
        blk, ro = h // 2, 64 * (h % 2) + 1
        Wcat[blk, ro:ro + 32] = W["Wo"][32 * h:32 * h + 32]
        for e in range(3):
            for p in range(4):
                Wcat[blk, ro + 32 + 4 * e + p] = Wpo4[h, p, e]

    bo_col = W["bo"] + W["bpo"] + W["bv"] @ W["Wo"]  # [C]

    # v-point offsets: tbv[n, 48e+4h+p] = sum_d bvp[h,p,d] rot[n,d,e] + trans[n,e]
    bvp3 = W["bvp"].reshape(H, 4, 3)
    tbv = np.einsum('hpd,nde->nehp', bvp3, rot[0]) + trans[0][:, :, None, None]
    VBm = np.zeros((128, VBW), np.float32)
    VBm[:, VROT_O:VROT_O + 72] = rot[0].reshape(NB, 128, 9).transpose(1, 0, 2).reshape(128, 72)
    VBm[:, VTBV_O:] = tbv.reshape(NB, 128, 144).transpose(1, 0, 2).reshape(128, 1152)

    shared = {
        "WB": WBm,
        "WPB": np.concatenate(
            [W["Wpb"], np.eye(128, dtype=np.float32)], axis=1).astype(BF),
        "VB": VBm,
    }
    for g in range(3):
        shared[f"KB{g}"] = rb_k[g].astype(BF)
        shared[f"PB{g}"] = np.ascontiguousarray(
            np.concatenate([bkp3[g], bqp3[g]], axis=1))

    in_maps = []
    for c in range(NCORES):
        m0 = c * M
        im = dict(shared)
        # pair [c, n, m] -> [c, j, m, nw]
        pT = pair[0, m0:m0 + M].transpose(2, 1, 0).reshape(128, NB, BL, M)
        pT = pT.transpose(0, 1, 3, 2).reshape(128, NB, M * BL)
        im["pairT8"] = np.ascontiguousarray(pT).astype(E4M3)

        # SF: [128, 3, N + M]: single^T full | single^T m-slice
        sfm = np.concatenate([single[0].T, single[0, m0:m0 + M].T], axis=1)  # [C, N+M]
        im["SF"] = np.ascontiguousarray(
            sfm.reshape(3, 128, N + M).transpose(1, 0, 2)
            .reshape(128, 3 * (N + M))).astype(BF)

        # QBLOB per g: rb 3*128 | tb 128, m-slice rotations
        rb_q = np.zeros((3, 128, QBW), np.float32)
        for g in range(3):
            for hh in range(4):
                for e in range(3):
                    r = 32 * hh + 4 * e
                    for d in range(3):
                        rb_q[g, r:r + 4, QRB_O + d * M:QRB_O + (d + 1) * M] = \
                            rot[0, m0:m0 + M, d, e]
                    rb_q[g, r:r + 4, QTB_O:QTB_O + M] = trans[0, m0:m0 + M, e] * SCALE
        for g in range(3):
            im[f"QB{g}"] = rb_q[g].astype(BF)

        miscm = np.zeros((128, MISC_W), np.float32)
        miscm[:, IDN_O:IDN_O + 128] = np.eye(128, dtype=np.float32)
        miscm[:, QEI_O:QEI_O + 768] = _qe_init()
        miscm[:, BSCK_O:BSCK_O + H] = _bsc(W["bk"])
        miscm[:, BSCQ_O:BSCQ_O + H] = _bsc(W["bq"] * SCALE)
        miscm[:, GAM_O:GAM_O + C] = np.broadcast_to(W["gamma"], (128, C))
        miscm[:, BET_O:BET_O + C] = np.broadcast_to(W["beta"], (128, C))
        miscm[:, SM_O:SM_O + C] = single[0, m0:m0 + M]
        miscm[:, WCAT_O:WCAT_O + 6 * C] = Wcat.transpose(1, 0, 2).reshape(128, 6 * C)
        miscm[:, BO_O:BO_O + 3] = bo_col.reshape(3, 128).T
        im["MISC"] = miscm
        in_maps.append(im)
    if _MERGE:
        ob, of = _bf_offsets(), _f32_offsets()
        merged = []
        for im in in_maps:
            bfb = np.zeros((128, _BF_W), BF)
            f32b = np.zeros((128, _F32_W), np.float32)
            for k, arr in im.items():
                if k == "pairT8":
                    continue
                if arr.dtype == BF:
                    bfb[:, ob[k]:ob[k] + arr.shape[1]] = arr
                else:
                    f32b[:, of[k]:of[k] + arr.shape[1]] = arr
            merged.append({"pairT8": im["pairT8"], "BFB": bfb, "F32B": f32b})
        return merged
    return in_maps


_NC_CACHE = {}


def get_nc():
    if "nc" not in _NC_CACHE:
        _NC_CACHE["nc"] = _build_program()
    return _NC_CACHE["nc"]


def kernel(**inputs) -> np.ndarray:
    mask = np.asarray(inputs["mask"])
    assert mask.all(), "kernel assumes all-ones mask"
    nc = get_nc()
    in_maps = _host_prep(inputs)
    res = run_bass_kernel_spmd(nc, in_maps, core_ids=list(range(NCORES)))
    out = np.concatenate([np.asarray(res.results[c]["out"]) for c in range(NCORES)], axis=0)
    return out.reshape(1, N, C).astype(np.float32)



# revision 53
# speedup vs baseline: 8.7700x; 4.7781x over previous
"""IPA (invariant point attention) Trainium2 kernel, 8-core SPMD, query-row sharded.

v3m: v3 flash pipeline with all inputs merged into 3 dram params.
  - launch overhead is ~45-60us PER INPUT PARAM through the axon/PJRT
    custom-call path (measured: 14 params ~645us extra vs 3 params ~0);
    all weights/constants ship as one bf16 blob + one f32 blob + pair fp8,
    and the unused partition_id input is disabled
  - v3 body: flash-style pipeline with transposed logits, on-chip rotation.
  - pair host-transposed to [c, block, m, n_within] fp8(e4m3); Wpb bf16
  - bias matmuls produce [n_block, 12h] tiles (n on partitions), staged per
    block in bf16 (h-major); logits computed transposed L^T[n, m] via one
    matmul + an identity-matmul that accumulates the staged bias into the
    same PSUM group; exp gives E^T directly; attn@v accumulates [45ch, m]
    in PSUM across blocks with a ones-channel (channel 0) giving softmax
    denominators; normalized at the end via a reciprocal outer-product
  - point projections emit rows padded to 32-row spacing per head and
    duplicated per rotation coordinate (h,e,p), so rotation = row-wise
    multiply-add against per-row broadcast tensors, and rotated features
    land in ke64/qe64 via 32-aligned on-chip DVE copies (no DRAM roundtrip)
  - v projections emitted token-major directly; v-point rotation via fused
    scalar_tensor_tensor with per-token rot scalars on partitions
  - q2 term dropped (softmax-invariant); pair-bias bias bpb dropped
    (softmax-invariant); bv folded into bo_col; scale folded into weights
"""
import os
import sys
sys.path.insert(0, '/opt/trn_rl_repo')

import numpy as np
import ml_dtypes

_ABLATE = set(os.environ.get("IPA_ABLATE", "").split(","))
_REPEAT = int(os.environ.get("IPA_REPEAT", "1"))
_MERGE = os.environ.get("IPA_MERGE", "1") == "1"
_DMA0 = os.environ.get("IPA_DMA0", "0") == "1"
_ONEPARAM = os.environ.get("IPA_ONEPARAM", "1") == "1"

import concourse.bass as bass
import concourse.mybir as mybir
from concourse.tile import TileContext
from concourse.vector_clock import ScopedClock
from concourse.bass_utils import run_bass_kernel_spmd

F32 = mybir.dt.float32
F16 = mybir.dt.float16
BF16 = mybir.dt.bfloat16
F8 = mybir.dt.float8e4
BF = ml_dtypes.bfloat16
E4M3 = ml_dtypes.float8_e4m3

N = 1024
M = 128
C = 384
H = 12
NB = 8          # n blocks
BL = 128        # block length
NCORES = 8
SCALE = (C // H) ** -0.5
EPS = 1e-5
Exp = mybir.ActivationFunctionType.Exp
Identity = mybir.ActivationFunctionType.Identity
Sqrt = mybir.ActivationFunctionType.Sqrt
Square = mybir.ActivationFunctionType.Square
ADD = mybir.AluOpType.add
MULT = mybir.AluOpType.mult

# WBLOB column offsets (per 128-row c-chunk)
WQ_O, WK_O, WKP_O, WQP_O, WVC_O = 0, 384, 768, 1920, 3072
WCH = 3600
# MISC column offsets
IDN_O, QEI_O, BSCK_O, BSCQ_O, GAM_O, BET_O, SM_O, WCAT_O, BO_O = \
    0, 128, 896, 908, 920, 1304, 1688, 2072, 4376
MISC_W = 4379
# KBLOB (per g, 128 rows): rb 3*1024 | tb 1024 | sel 12
KRB_O, KTB_O, KSEL_O = 0, 3072, 4096
KBW = 4108
# QBLOB (per g, 128 rows): rb 3*128 | tb 128
QRB_O, QTB_O = 0, 384
QBW = 512
# VBLOB: rot9 (8*9) | tbv (8*144)
VROT_O, VTBV_O = 0, 72
VBW = 72 + 1152

_MAXW = int(os.environ.get("IPA_MAXW", "1"))


def _patched_drain_and_barrier(self, tick_clock, wait_clock):
    # walrus rejects >2 sync waits on one Drain; split tail waits across nops
    nc = self.nc
    probe = nc.sync.nop()
    wait_clock.add_sem_waits(probe.ins, ScopedClock({None: tick_clock.global_clock}))
    waits = list(probe.ins.sync_info.on_wait or [])
    probe.ins.sync_info.on_wait = waits[:_MAXW]
    rest = waits[_MAXW:]
    while rest:
        n2 = nc.sync.nop()
        n2.ins.sync_info = mybir.SyncInfo(on_wait=rest[:_MAXW], on_update=[])
        rest = rest[_MAXW:]
    nc.sync.drain()
    nc.all_engine_barrier()
    assert self.sems is not None
    popped = nc._tile_sem_poison_stack.pop()
    assert popped is self._sem_poison
    nc.clear_and_free_semaphores(list(self.sems.allocated().values()))
    nc.all_engine_barrier()


TileContext._drain_and_barrier = _patched_drain_and_barrier

_orig_lower_ordered = TileContext._lower_ordered_insts


def _split_waits_then_lower(self, ordered):
    # HW instructions encode a limited number of sync waits; hoist excess
    # onto NoOps inserted immediately before, on the same engine.
    nc = self.nc
    for bb in list(ordered.keys()):
        insts = ordered[bb]
        new = []
        for inst in insts:
            si = getattr(inst, "sync_info", None)
            if si is not None and si.on_wait and len(si.on_wait) > _MAXW:
                waits = list(si.on_wait)
                while len(waits) > _MAXW:
                    chunk, waits = waits[:_MAXW], waits[_MAXW:]
                    nop = mybir.InstNoOp(
                        name=nc.get_next_instruction_name(),
                        engine=inst.engine, ins=[], outs=[], bass_nofuse=True,
                        sync_info=mybir.SyncInfo(on_wait=chunk, on_update=[]))
                    new.append(nop)
                si.on_wait = waits
            new.append(inst)
        ordered[bb] = new
    return _orig_lower_ordered(self, ordered)


TileContext._lower_ordered_insts = _split_waits_then_lower


def _emit(nc, pairT8, WB, SF, MISC, VB, KB, QB, PB, WPB, OUT):
    with TileContext(nc) as tc:
        with tc.tile_pool(name="persist", bufs=1) as pp, \
             tc.tile_pool(name="pair", bufs=2) as pairp, \
             tc.tile_pool(name="dstage", bufs=1, space="DRAM") as dstp:
            qe64 = pp.tile([128, 6 * M], BF16)
            ke64 = pp.tile([128, 6 * N], BF16)
            v_nat = pp.tile([128, NB * 540], BF16)
            stage = pp.tile([128, NB * 1536], BF16)
            wpb_sb = pp.tile([128, H + 128], BF16)
            misc = pp.tile([128, MISC_W], F32)
            vb_sb = pp.tile([128, VBW], F32)
            ones44 = pp.tile([1, 64], F32)
            cat_sb = pp.tile([128, 6 * 128], F32)
            k2sb = pp.tile([H, N], F32)
            k2_dram = dstp.tile([H, N], F32)

            nc.sync.dma_start(wpb_sb[:], WPB[:])
            nc.sync.dma_start(misc[:], MISC[:])
            nc.sync.dma_start(vb_sb[:], VB[:])
            nc.vector.memset(ke64[:], 0.0)
            nc.vector.memset(ones44[:], 1.0)
            nc.vector.memset(cat_sb[:], 0.0)
            nc.vector.tensor_copy(qe64[:], misc[:, QEI_O:QEI_O + 768])
            v_nat_v = v_nat[:].rearrange("p (j h c) -> p j h c", j=NB, h=H)
            nc.vector.memset(v_nat_v[:, :, :, 0:1], 1.0)

            idn_sb = misc[:, IDN_O:IDN_O + 128]
            idnb = wpb_sb[:, H:H + 128]

            # ====== phase A ======
            with tc.tile_pool(name="wload", bufs=1) as wl, \
                 tc.tile_pool(name="rot", bufs=2) as rp:
                wb = wl.tile([128, 3 * WCH], BF16)
                nc.sync.dma_start(wb[:], WB[:])
                sf = wl.tile([128, 3 * (N + M)], BF16)
                nc.sync.dma_start(sf[:], SF[:])
                kb = []
                qb = []
                pb = []
                for g in range(3):
                    t = wl.tile([128, KBW], BF16, tag=f"kb{g}", name=f"kb{g}")
                    nc.sync.dma_start(t[:], KB[g][:])
                    kb.append(t)
                    t = wl.tile([128, QBW], BF16, tag=f"qb{g}", name=f"qb{g}")
                    nc.sync.dma_start(t[:], QB[g][:])
                    qb.append(t)
                    t = wl.tile([128, 6], F32, tag=f"pb{g}", name=f"pb{g}")
                    nc.sync.dma_start(t[:], PB[g][:])
                    pb.append(t)
                kpT3 = [wl.tile([128, 3 * N], BF16, tag=f"kpT{g}", name=f"kpT{g}")
                        for g in range(3)]
                qpT3 = [wl.tile([128, 3 * M], BF16, tag=f"qpT{g}", name=f"qpT{g}")
                        for g in range(3)]
                vtmp = wl.tile([128, NB * 144], F32)

                def sfT(ci, o, w):
                    return sf[:, ci * (N + M) + o: ci * (N + M) + o + w]

                def smT(ci, o, w):
                    return sf[:, ci * (N + M) + N + o: ci * (N + M) + N + o + w]

                with tc.tile_pool(name="p1ps", bufs=2, space="PSUM") as pps, \
                     tc.tile_pool(name="vps", bufs=2, space="PSUM") as vps, \
                     tc.tile_pool(name="k2ps", bufs=1, space="PSUM") as k2pool:
                    # ---- scalar q/k projections (feature-major into ke64/qe64) ----
                    def grp_scal(wo, dste, bsc_o, mov, width, co, nb):
                        o = nb * 512
                        w = min(512, width - o)
                        ps = pps.tile([128, 512], F32, tag="proj", name="ps")
                        for ci in range(3):
                            nc.tensor.matmul(
                                ps[:, 0:w],
                                wb[:, ci * WCH + wo + co * 128: ci * WCH + wo + co * 128 + 128],
                                mov(ci, o, w),
                                start=(ci == 0), stop=(ci == 2))
                        for hh in range(4):
                            h = 4 * co + hh
                            t, u = h // 2, h % 2
                            nc.scalar.activation(
                                dste[64 * u:64 * u + 32, t * width + o: t * width + o + w],
                                ps[32 * hh:32 * hh + 32, 0:w], Identity,
                                bias=misc[64 * u:64 * u + 32, bsc_o + h:bsc_o + h + 1])

                    for co in range(3):
                        for nb in range(2):
                            grp_scal(WK_O, ke64, BSCK_O, sfT, N, co, nb)
                    for co in range(3):
                        grp_scal(WQ_O, qe64, BSCQ_O, smT, M, co, 0)

                    # ---- point projections, rows 32*hh + 4e + p per g of 4 heads ----
                    def grp_pt3(wo, dst, pbt, dcol, mov, width, g, d, nb):
                        o = nb * 512
                        w = min(512, width - o)
                        ps = pps.tile([128, 512], F32, tag="proj", name="ps")
                        for ci in range(3):
                            nc.tensor.matmul(
                                ps[:, 0:w],
                                wb[:, ci * WCH + wo + d * 384 + 128 * g:
                                   ci * WCH + wo + d * 384 + 128 * g + 128],
                                mov(ci, o, w),
                                start=(ci == 0), stop=(ci == 2))
                        nc.scalar.activation(
                            dst[:, d * width + o: d * width + o + w],
                            ps[:, 0:w], Identity, bias=pbt[:, dcol + d:dcol + d + 1])

                    for g in range(3):
                        for d in range(3):
                            for nb in range(2):
                                grp_pt3(WKP_O, kpT3[g], pb[g], 0, sfT, N, g, d, nb)
                    for g in range(3):
                        for d in range(3):
                            grp_pt3(WQP_O, qpT3[g], pb[g], 3, smT, M, g, d, 0)

                    # ---- v projections: token-major direct ----
                    for j in range(NB):
                        vs_ps = vps.tile([128, 384], F32, tag="vs", name="vs_ps")
                        vp_ps = vps.tile([128, 144], F32, tag="vp", name="vp_ps")
                        for ci in range(3):
                            nc.tensor.matmul(
                                vs_ps[:],
                                sfT(ci, j * BL, BL),
                                wb[:, ci * WCH + WVC_O: ci * WCH + WVC_O + 384],
                                start=(ci == 0), stop=(ci == 2))
                            nc.tensor.matmul(
                                vp_ps[:],
                                sfT(ci, j * BL, BL),
                                wb[:, ci * WCH + WVC_O + 384: ci * WCH + WVC_O + 528],
                                start=(ci == 0), stop=(ci == 2))
                        nc.scalar.copy(
                            v_nat_v[:, j, :, 1:33],
                            vs_ps[:].rearrange("p (h c) -> p h c", h=H))
                        nc.scalar.copy(vtmp[:, 144 * j:144 * (j + 1)], vp_ps[:])

                    # ---- k rotation on-chip: rc = sum_d kp_d * rb_d + tb ----
                    RCH = 512
                    k2t = [k2pool.tile([H, 512], F32, tag=f"k2_{half}", name=f"k2t{half}")
                           for half in range(2)]

                    def rot_chain(src, blob, rbo, tbo, width, o, w, tag):
                        rc = rp.tile([128, RCH], BF16, tag=f"rc{tag}", name="rc")
                        t2 = rp.tile([128, RCH], BF16, tag=f"rt{tag}", name="t2")
                        nc.vector.tensor_tensor(
                            rc[:, 0:w], src[:, o:o + w],
                            blob[:, rbo + o:rbo + o + w], MULT)
                        nc.vector.tensor_tensor(
                            t2[:, 0:w], src[:, width + o:width + o + w],
                            blob[:, rbo + width + o:rbo + width + o + w], MULT)
                        nc.vector.tensor_tensor(rc[:, 0:w], rc[:, 0:w], t2[:, 0:w], ADD)
                        nc.vector.tensor_tensor(
                            t2[:, 0:w], src[:, 2 * width + o:2 * width + o + w],
                            blob[:, rbo + 2 * width + o:rbo + 2 * width + o + w], MULT)
                        nc.vector.tensor_tensor(rc[:, 0:w], rc[:, 0:w], t2[:, 0:w], ADD)
                        nc.vector.tensor_tensor(
                            rc[:, 0:w], rc[:, 0:w],
                            blob[:, tbo + o:tbo + o + w], ADD)
                        return rc

                    for g in range(3):
                        for ci_ in range(2):
                            o = ci_ * RCH
                            rc = rot_chain(kpT3[g], kb[g], KRB_O, KTB_O, N, o, RCH, "k")
                            sq = rp.tile([128, RCH], BF16, tag="sq", name="sq")
                            nc.vector.tensor_tensor(sq[:], rc[:], rc[:], MULT)
                            nc.tensor.matmul(
                                k2t[ci_][:], kb[g][:, KSEL_O:KSEL_O + 12], sq[:],
                                start=(g == 0), stop=(g == 2))
                            for hh in range(4):
                                h = 4 * g + hh
                                t, u = h // 2, h % 2
                                nc.vector.tensor_copy(
                                    ke64[64 * u + 32:64 * u + 44, t * N + o: t * N + o + RCH],
                                    rc[32 * hh:32 * hh + 12, :])
                        qrc = rot_chain(qpT3[g], qb[g], QRB_O, QTB_O, M, 0, M, "q")
                        for hh in range(4):
                            h = 4 * g + hh
                            t, u = h // 2, h % 2
                            nc.vector.tensor_copy(
                                qe64[64 * u + 32:64 * u + 44, t * M:(t + 1) * M],
                                qrc[32 * hh:32 * hh + 12, 0:M])

                    # k2 rows into ke64 (partition scatter via DRAM roundtrip)
                    for half in range(2):
                        nc.vector.tensor_copy(
                            k2sb[:, half * 512:(half + 1) * 512], k2t[half][:])
                    nc.gpsimd.dma_start(k2_dram[:], k2sb[:])
                    for u in range(2):
                        dst = ke64[64 * u + 44: 64 * u + 45, :].rearrange(
                            "one (t n) -> one t n", t=6)
                        src = k2_dram[:].rearrange("(t u2) n -> u2 t n", u2=2)[u:u + 1]
                        nc.gpsimd.dma_start(dst, src)

                    # ---- v-point rotation via per-token scalars ----
                    rot9 = vb_sb[:, VROT_O:VROT_O + 72]
                    tbv = vb_sb[:, VTBV_O:VTBV_O + 1152]
                    for j in range(NB):
                        for e in range(3):
                            t1 = rp.tile([128, 48], F32, tag="vst1", name="t1")
                            t2 = rp.tile([128, 48], F32, tag="vst2", name="t2")
                            nc.vector.scalar_tensor_tensor(
                                t1[:], vtmp[:, 144 * j:144 * j + 48],
                                rot9[:, 9 * j + e:9 * j + e + 1],
                                tbv[:, 144 * j + 48 * e: 144 * j + 48 * e + 48], MULT, ADD)
                            nc.vector.scalar_tensor_tensor(
                                t2[:], vtmp[:, 144 * j + 48:144 * j + 96],
                                rot9[:, 9 * j + 3 + e:9 * j + 4 + e],
                                t1[:], MULT, ADD)
                            nc.vector.scalar_tensor_tensor(
                                v_nat_v[:, j, :, 33 + 4 * e:37 + 4 * e],
                                vtmp[:, 144 * j + 96:144 * j + 144].rearrange(
                                    "p (h c) -> p h c", h=H),
                                rot9[:, 9 * j + 6 + e:9 * j + 7 + e],
                                t2[:].rearrange("p (h c) -> p h c", h=H), MULT, ADD)

            # ===== interleaved bias stream + flash attention =====
            stage_v2 = stage[:].rearrange("p (j h m) -> p j h m", j=NB, h=H)
            with tc.tile_pool(name="accps", bufs=1, space="PSUM") as accp:
                accT = [accp.tile([128, 512], F32, tag=f"acc{i}", name=f"acc{i}")
                        for i in range(3)]
                with tc.tile_pool(name="biasps", bufs=2, space="PSUM") as bps_pool, \
                     tc.tile_pool(name="lps", bufs=3, space="PSUM") as lpool, \
                     tc.tile_pool(name="esb", bufs=2) as esb:
                    ptj_tiles = []
                    for j in range(2):
                        ptj = pairp.tile([128, M * BL], F8, tag="pair", name="ptj")
                        nc.sync.dma_start(ptj[:], pairT8[:, 0 if _DMA0 else j])
                        ptj_tiles.append(ptj)

                    def bias_block(j):
                        if j >= 2:
                            ptj = pairp.tile([128, M * BL], F8, tag="pair", name="ptj")
                            if "nodma" not in _ABLATE:
                                nc.sync.dma_start(ptj[:], pairT8[:, 0 if _DMA0 else j])
                        else:
                            ptj = ptj_tiles[j]
                        if "nobias" in _ABLATE:
                            return
                        for g in range(4):
                            bps = bps_pool.tile([128, 384], F32, tag="bias", name="bps")
                            for mm in range(32):
                                m = 32 * g + mm
                                nc.tensor.matmul(
                                    bps[:, 12 * mm:12 * mm + 12],
                                    ptj[:, BL * m: BL * m + BL], wpb_sb[:, 0:H],
                                    start=True, stop=True)
                            nc.scalar.copy(
                                stage_v2[:, j, :, 32 * g:32 * g + 32],
                                bps[:].rearrange("p (m h) -> p h m", h=H))

                    def flash_block(j):
                        E = esb.tile([128, H * 128], BF16, tag="E", name="E")
                        for hg in range(3):
                            lps = lpool.tile([128, 512], F32, tag="lps", name="lps")
                            for hh in range(4):
                                h = 4 * hg + hh
                                t, u = h // 2, h % 2
                                nc.tensor.matmul(
                                    lps[:, 128 * hh:128 * hh + 128],
                                    ke64[64 * u:64 * u + 64, t * N + j * BL: t * N + j * BL + BL],
                                    qe64[64 * u:64 * u + 64, t * M:(t + 1) * M],
                                    start=(hh == 0), stop=False)
                                nc.tensor.matmul(
                                    lps[:, 128 * hh:128 * hh + 128], idnb,
                                    stage[:, j * 1536 + 128 * h: j * 1536 + 128 * h + 128],
                                    start=False, stop=(hh == 3))
                            nc.scalar.activation(
                                E[:, 512 * hg:512 * (hg + 1)], lps[:], Exp)
                        for h in range(H):
                            nc.tensor.matmul(
                                accT[h // 4][0:45, (h % 4) * 128:(h % 4) * 128 + 128],
                                v_nat[:, 540 * j + 45 * h: 540 * j + 45 * h + 45],
                                E[:, h * 128:(h + 1) * 128],
                                start=(j == 0 and h % 4 == 0),
                                stop=(j == NB - 1 and h % 4 == 3))

                    if "nobias" in _ABLATE:
                        nc.vector.memset(stage[:], 0.0)
                    for j in range(NB):
                        bias_block(j)
                        if j >= 2 and "noflash" not in _ABLATE:
                            flash_block(j - 2)
                    if "noflash" not in _ABLATE:
                        flash_block(NB - 2)
                        flash_block(NB - 1)
                    else:
                        for i in range(3):
                            nc.vector.memset(accT[i][:], 1.0)
                # normalize: cat[ch, m] = acc[ch, m] * (1/den[m]) via outer product
                # den is channel 0; row 0 of cat carries junk (Wcat row is zero)
                with tc.tile_pool(name="obps", bufs=2, space="PSUM") as obp, \
                     tc.tile_pool(name="fin", bufs=2) as fin:
                    for h in range(H):
                        at = accT[h // 4]
                        c0 = (h % 4) * 128
                        rcp = fin.tile([1, 128], F32, tag="rcp", name="rcp")
                        nc.vector.reciprocal(rcp[:], at[0:1, c0:c0 + 128])
                        obc = obp.tile([45, 128], F32, tag="obc", name="obc")
                        nc.tensor.matmul(obc[:], ones44[:, 0:45], rcp[:],
                                         start=True, stop=True)
                        obs = fin.tile([45, 128], F32, tag="obs", name="obs")
                        nc.scalar.copy(obs[:], obc[:])
                        nc.vector.tensor_tensor(
                            cat_sb[64 * (h % 2):64 * (h % 2) + 45, (h // 2) * 128:(h // 2 + 1) * 128],
                            at[0:45, c0:c0 + 128], obs[:], MULT)

            # ============ output projection + residual + LN ============
            with tc.tile_pool(name="fin_sb", bufs=1) as fsb_pool, \
                 tc.tile_pool(name="finps", bufs=1, space="PSUM") as fpool, \
                 tc.tile_pool(name="tps", bufs=2, space="PSUM") as tpool:
                fps = fpool.tile([128, C], F32)
                for b in range(3):
                    for k in range(6):
                        nc.tensor.matmul(
                            fps[:, b * 128:(b + 1) * 128],
                            misc[:, WCAT_O + k * C + b * 128: WCAT_O + k * C + b * 128 + 128],
                            cat_sb[:, k * 128:(k + 1) * 128],
                            start=(k == 0), stop=(k == 5))
                fsb = fsb_pool.tile([128, C], F32)
                for b in range(3):
                    nc.scalar.activation(
                        fsb[:, b * 128:(b + 1) * 128], fps[:, b * 128:(b + 1) * 128],
                        Identity, bias=misc[:, BO_O + b:BO_O + b + 1])
                xres = fsb_pool.tile([128, C], F32)
                for b in range(3):
                    tp = tpool.tile([128, 128], F32)
                    nc.tensor.transpose(tp[:], fsb[:, b * 128:(b + 1) * 128], idn_sb)
                    nc.vector.tensor_tensor(
                        xres[:, b * 128:(b + 1) * 128], tp[:],
                        misc[:, SM_O + b * 128:SM_O + (b + 1) * 128], ADD)
                mu = fsb_pool.tile([128, 1], F32)
                nc.vector.reduce_sum(mu[:], xres[:], axis=mybir.AxisListType.X)
                nc.scalar.mul(mu[:], mu[:], 1.0 / C)
                xc = fsb_pool.tile([128, C], F32)
                nc.vector.tensor_scalar_sub(xc[:], xres[:], mu[:])
                x2 = fsb_pool.tile([128, C], F32)
                var_r = fsb_pool.tile([128, 1], F32)
                nc.scalar.activation(x2[:], xc[:], Square, accum_out=var_r[:])
                epsc = fsb_pool.tile([128, 1], F32)
                nc.vector.memset(epsc[:], EPS)
                stdc = fsb_pool.tile([128, 1], F32)
                nc.scalar.activation(stdc[:], var_r[:], Sqrt, scale=1.0 / C, bias=epsc[:])
                rstd = fsb_pool.tile([128, 1], F32)
                nc.vector.reciprocal(rstd[:], stdc[:])
                xg = fsb_pool.tile([128, C], F32)
                nc.vector.scalar_tensor_tensor(
                    xg[:], xc[:], rstd[:], misc[:, GAM_O:GAM_O + C], MULT, MULT)
                osb = fsb_pool.tile([128, C], F32)
                nc.vector.tensor_tensor(osb[:], xg[:], misc[:, BET_O:BET_O + C], ADD)
                nc.sync.dma_start(OUT[:], osb[:])



# merged-blob column offsets (bf16 blob / f32 blob)
_BF_W = 3 * WCH + 3 * (N + M) + 3 * KBW + 3 * QBW + (H + 128)
_F32_W = MISC_W + VBW + 3 * 6


def _bf_offsets():
    o = {}
    c = 0
    o["WB"] = c; c += 3 * WCH
    o["SF"] = c; c += 3 * (N + M)
    for g in range(3):
        o[f"KB{g}"] = c; c += KBW
    for g in range(3):
        o[f"QB{g}"] = c; c += QBW
    o["WPB"] = c; c += H + 128
    assert c == _BF_W
    return o


def _f32_offsets():
    o = {}
    c = 0
    o["MISC"] = c; c += MISC_W
    o["VB"] = c; c += VBW
    for g in range(3):
        o[f"PB{g}"] = c; c += 6
    assert c == _F32_W
    return o


def _build_program():
    nc = bass.Bass(enable_partition_id=False)
    dp = nc.declare_dram_parameter

    if _ONEPARAM:
        TOT = _BF_W + 2 * _F32_W + NB * (M * BL) // 2
        BLOB = dp("blob", [128, TOT], BF16, isOutput=False)
        ob, of = _bf_offsets(), _f32_offsets()

        def bsl(k, w):
            return BLOB[:, ob[k]:ob[k] + w]

        def fsl(k, w):
            a = _BF_W + 2 * of[k]
            return BLOB[:, a:a + 2 * w].bitcast(F32)

        class _PairShim:
            def __getitem__(self, idx):
                j = idx[1]
                o = _BF_W + 2 * _F32_W + j * (M * BL) // 2
                return BLOB[:, o:o + (M * BL) // 2].bitcast(F8)

        pairT8 = _PairShim()
        WB = bsl("WB", 3 * WCH)
        SF = bsl("SF", 3 * (N + M))
        KB = [bsl(f"KB{g}", KBW) for g in range(3)]
        QB = [bsl(f"QB{g}", QBW) for g in range(3)]
        WPB = bsl("WPB", H + 128)
        MISC = fsl("MISC", MISC_W)
        VB = fsl("VB", VBW)
        PB = [fsl(f"PB{g}", 6) for g in range(3)]
        OUT = dp("out", [M, C], F32, isOutput=True)
        for _rep in range(_REPEAT):
            _emit(nc, pairT8, WB, SF, MISC, VB, KB, QB, PB, WPB, OUT)
        return nc

    pairT8 = dp("pairT8", [128, NB, M * BL], F8, isOutput=False)  # [c, j, m*128+nw]
    if _MERGE:
        BFB = dp("BFB", [128, _BF_W], BF16, isOutput=False)
        F32B = dp("F32B", [128, _F32_W], F32, isOutput=False)
        ob, of = _bf_offsets(), _f32_offsets()

        def bsl(k, w):
            return BFB[:, ob[k]:ob[k] + w]

        def fsl(k, w):
            return F32B[:, of[k]:of[k] + w]

        WB = bsl("WB", 3 * WCH)
        SF = bsl("SF", 3 * (N + M))
        KB = [bsl(f"KB{g}", KBW) for g in range(3)]
        QB = [bsl(f"QB{g}", QBW) for g in range(3)]
        WPB = bsl("WPB", H + 128)
        MISC = fsl("MISC", MISC_W)
        VB = fsl("VB", VBW)
        PB = [fsl(f"PB{g}", 6) for g in range(3)]
    else:
        WB = dp("WB", [128, 3 * WCH], BF16, isOutput=False)
        SF = dp("SF", [128, 3 * (N + M)], BF16, isOutput=False)
        MISC = dp("MISC", [128, MISC_W], F32, isOutput=False)
        VB = dp("VB", [128, VBW], F32, isOutput=False)
        KB = [dp(f"KB{g}", [128, KBW], BF16, isOutput=False) for g in range(3)]
        QB = [dp(f"QB{g}", [128, QBW], BF16, isOutput=False) for g in range(3)]
        PB = [dp(f"PB{g}", [128, 6], F32, isOutput=False) for g in range(3)]
        WPB = dp("WPB", [128, H + 128], BF16, isOutput=False)  # wpb | idn(bf16)
    OUT = dp("out", [M, C], F32, isOutput=True)

    for _rep in range(_REPEAT):
        _emit(nc, pairT8, WB, SF, MISC, VB, KB, QB, PB, WPB, OUT)
    return nc


def _bsc(b):
    out = np.zeros((128, H), np.float32)
    for h in range(H):
        u = h % 2
        out[64 * u:64 * u + 32, h] = b[32 * h:32 * h + 32]
    return out


def _qe_init():
    q = np.zeros((128, 6 * 128), np.float32)
    q[44, :] = 1.0
    q[108, :] = 1.0
    return q


def _host_prep(inputs):
    single = np.asarray(inputs["single"], np.float32)
    pair = np.asarray(inputs["pair"], np.float32)
    rot = np.asarray(inputs["rot"], np.float32)
    trans = np.asarray(inputs["trans"], np.float32)
    W = {k: np.asarray(inputs[k], np.float32) for k in
         ["Wq", "bq", "Wk", "bk", "Wv", "bv", "Wpb", "bpb", "Wqp", "bqp",
          "Wkp", "bkp", "Wvp", "bvp", "Wo", "bo", "Wpo", "bpo", "gamma", "beta"]}

    # point weights, rows 32*hh + 4e + p per g-tile of 4 heads (h = 4g + hh)
    def perm_dup(Wp, scale):
        W4 = Wp.reshape(C, H, 4, 3) * scale          # [c, h, p, d]
        out = np.zeros((C, 3, 3, 128), np.float32)   # [c, d, g, row]
        for h in range(H):
            g, hh = h // 4, h % 4
            for e in range(3):
                out[:, :, g, 32 * hh + 4 * e: 32 * hh + 4 * e + 4] = \
                    W4[:, h].transpose(0, 2, 1)      # [c, d, p]
        return out.reshape(C, 3 * 384)

    Wkp3 = perm_dup(W["Wkp"], 1.0)
    Wqp3 = perm_dup(W["Wqp"], SCALE)

    def pb_cols(bp, scale):
        b4 = bp.reshape(H, 4, 3) * scale             # [h, p, d]
        out = np.zeros((3, 128, 3), np.float32)      # [g, row, d]
        for h in range(H):
            g, hh = h // 4, h % 4
            for e in range(3):
                out[g, 32 * hh + 4 * e: 32 * hh + 4 * e + 4, :] = b4[h]
        return out

    bkp3 = pb_cols(W["bkp"], 1.0)
    bqp3 = pb_cols(W["bqp"], SCALE)

    # v combined: scalar [C,384] | points [C,144] in (d, h, p) order
    Wvp_d = W["Wvp"].reshape(C, H, 4, 3).transpose(0, 3, 1, 2).reshape(C, 144)
    Wvc = np.concatenate([W["Wv"], Wvp_d], axis=1)   # [C, 528]

    # WBLOB [128, 3, WCH]
    Wq_s = W["Wq"] * SCALE
    wcat_all = np.concatenate([Wq_s, W["Wk"], Wkp3, Wqp3, Wvc], axis=1)  # [C, WCH]
    WBm = np.ascontiguousarray(wcat_all.reshape(3, 128, WCH).transpose(1, 0, 2)
                               .reshape(128, 3 * WCH)).astype(BF)

    # KBLOB per g: rb[d] 3*1024 | tb 1024 | sel 12; rows 32hh + 4e + p
    rb_k = np.zeros((3, 128, KBW), np.float32)
    for g in range(3):
        for hh in range(4):
            h = 4 * g + hh
            for e in range(3):
                r = 32 * hh + 4 * e
                for d in range(3):
                    rb_k[g, r:r + 4, KRB_O + d * N:KRB_O + (d + 1) * N] = rot[0, :, d, e]
                rb_k[g, r:r + 4, KTB_O:KTB_O + N] = trans[0, :, e]
                rb_k[g, r:r + 4, KSEL_O + h] = -0.5 * SCALE

    # Wcat for output projection; cat rows 64u + 1 + i (row 64u is junk)
    Wcat = np.zeros((6, 128, C), np.float32)
    Wpo4 = W["Wpo"].reshape(H, 4, 3, C)
    for h in range(H):
        blk, ro = h // 2, 64 * (h % 2) + 1
        Wcat[blk, ro:ro + 32] = W["Wo"][32 * h:32 * h + 32]
        for e in range(3):
            for p in range(4):
                Wcat[blk, ro + 32 + 4 * e + p] = Wpo4[h, p, e]

    bo_col = W["bo"] + W["bpo"] + W["bv"] @ W["Wo"]  # [C]

    # v-point offsets: tbv[n, 48e+4h+p] = sum_d bvp[h,p,d] rot[n,d,e] + trans[n,e]
    bvp3 = W["bvp"].reshape(H, 4, 3)
    tbv = np.einsum('hpd,nde->nehp', bvp3, rot[0]) + trans[0][:, :, None, None]
    VBm = np.zeros((128, VBW), np.float32)
    VBm[:, VROT_O:VROT_O + 72] = rot[0].reshape(NB, 128, 9).transpose(1, 0, 2).reshape(128, 72)
    VBm[:, VTBV_O:] = tbv.reshape(NB, 128, 144).transpose(1, 0, 2).reshape(128, 1152)

    shared = {
        "WB": WBm,
        "WPB": np.concatenate(
            [W["Wpb"], np.eye(128, dtype=np.float32)], axis=1).astype(BF),
        "VB": VBm,
    }
    for g in range(3):
        shared[f"KB{g}"] = rb_k[g].astype(BF)
        shared[f"PB{g}"] = np.ascontiguousarray(
            np.concatenate([bkp3[g], bqp3[g]], axis=1))

    in_maps = []
    for c in range(NCORES):
        m0 = c * M
        im = dict(shared)
        # pair [c, n, m] -> [c, j, m, nw]
        pT = pair[0, m0:m0 + M].transpose(2, 1, 0).reshape(128, NB, BL, M)
        pT = pT.transpose(0, 1, 3, 2).reshape(128, NB, M * BL)
        im["pairT8"] = np.ascontiguousarray(pT).astype(E4M3)

        # SF: [128, 3, N + M]: single^T full | single^T m-slice
        sfm = np.concatenate([single[0].T, single[0, m0:m0 + M].T], axis=1)  # [C, N+M]
        im["SF"] = np.ascontiguousarray(
            sfm.reshape(3, 128, N + M).transpose(1, 0, 2)
            .reshape(128, 3 * (N + M))).astype(BF)

        # QBLOB per g: rb 3*128 | tb 128, m-slice rotations
        rb_q = np.zeros((3, 128, QBW), np.float32)
        for g in range(3):
            for hh in range(4):
                for e in range(3):
                    r = 32 * hh + 4 * e
                    for d in range(3):
                        rb_q[g, r:r + 4, QRB_O + d * M:QRB_O + (d + 1) * M] = \
                            rot[0, m0:m0 + M, d, e]
                    rb_q[g, r:r + 4, QTB_O:QTB_O + M] = trans[0, m0:m0 + M, e] * SCALE
        for g in range(3):
            im[f"QB{g}"] = rb_q[g].astype(BF)

        miscm = np.zeros((128, MISC_W), np.float32)
        miscm[:, IDN_O:IDN_O + 128] = np.eye(128, dtype=np.float32)
        miscm[:, QEI_O:QEI_O + 768] = _qe_init()
        miscm[:, BSCK_O:BSCK_O + H] = _bsc(W["bk"])
        miscm[:, BSCQ_O:BSCQ_O + H] = _bsc(W["bq"] * SCALE)
        miscm[:, GAM_O:GAM_O + C] = np.broadcast_to(W["gamma"], (128, C))
        miscm[:, BET_O:BET_O + C] = np.broadcast_to(W["beta"], (128, C))
        miscm[:, SM_O:SM_O + C] = single[0, m0:m0 + M]
        miscm[:, WCAT_O:WCAT_O + 6 * C] = Wcat.transpose(1, 0, 2).reshape(128, 6 * C)
        miscm[:, BO_O:BO_O + 3] = bo_col.reshape(3, 128).T
        im["MISC"] = miscm
        in_maps.append(im)
    if _MERGE or _ONEPARAM:
        ob, of = _bf_offsets(), _f32_offsets()
        merged = []
        for im in in_maps:
            bfb = np.zeros((128, _BF_W), BF)
            f32b = np.zeros((128, _F32_W), np.float32)
            for k, arr in im.items():
                if k == "pairT8":
                    continue
                if arr.dtype == BF:
                    bfb[:, ob[k]:ob[k] + arr.shape[1]] = arr
                else:
                    f32b[:, of[k]:of[k] + arr.shape[1]] = arr
            if _ONEPARAM:
                TOT = _BF_W + 2 * _F32_W + NB * (M * BL) // 2
                blob = np.zeros((128, TOT), BF)
                blob[:, 0:_BF_W] = bfb
                blob[:, _BF_W:_BF_W + 2 * _F32_W] = (
                    np.ascontiguousarray(f32b).view(np.uint16).view(BF))
                p8 = np.ascontiguousarray(im["pairT8"]).reshape(128, NB * M * BL)
                blob[:, _BF_W + 2 * _F32_W:] = p8.view(np.uint16).view(BF)
                merged.append({"blob": blob})
            else:
                merged.append({"pairT8": im["pairT8"], "BFB": bfb, "F32B": f32b})
        return merged
    return in_maps


_NC_CACHE = {}


def get_nc():
    if "nc" not in _NC_CACHE:
        _NC_CACHE["nc"] = _build_program()
    return _NC_CACHE["nc"]


def kernel(**inputs) -> np.ndarray:
    mask = np.asarray(inputs["mask"])
    assert mask.all(), "kernel assumes all-ones mask"
    nc = get_nc()
    in_maps = _host_prep(inputs)
    res = run_bass_kernel_spmd(nc, in_maps, core_ids=list(range(NCORES)))
    out = np.concatenate([np.asarray(res.results[c]["out"]) for c in range(NCORES)], axis=0)
    return out.reshape(1, N, C).astype(np.float32)

